# revision 42
# baseline (speedup 1.0000x reference)
"""Multi-head causal attention (QKV proj + attention + O proj) on 8 TRN2 cores.

Sharding: data-parallel over batch (4) x tensor-parallel over heads (2 groups
of 8 heads).  Core c handles batch c//2, head-group c%2.  Each core computes
its group's partial o_proj output; the host sums the two partials per batch.

Layout strategy (all activations arrive pre-transposed from the host, so the
kernel never transposes on-device):
  - qT, kT per head-pair M-tile: (128 head-dims, L) from  W.T-slice @ X.T
  - v natural (tokens, head-dims) with a fused ones-column for the softmax
    denominator: av_psum = v_aug.T @ P.T gives (65, Nq) where row 64 is the
    per-query sum of probabilities.
  - scores are computed transposed (keys on partitions, queries free); the two
    heads of an M-tile run as concurrent PE row-tiles (K=64 at row 0 / 64).
  - AV emission is software-pipelined one key-block behind scores/exp so the
    strict-FIFO PE queue never stalls waiting on the ACT-engine exp.
  - causal diagonal blocks: exp first, then one fused tensor_tensor multiply
    (both heads at once) against a host-duplicated triangular mask.
  - softmax denominators: fast-approx reciprocal on the 1-partition row (bf16),
    then a K=1 PE matmul (ones outer product) broadcasts it to partitions
    64..127 of a PSUM bank -- no DRAM bounce.
Schedule: the prologue DMAs are sliced per contraction tile and spread over
three engine queues in need-order (vector: weights, sync: x chunks, scalar:
tiny constants) so projection chains start within ~3us; later x chunks are
gated behind the critical phase.  All o_proj work is deferred to the last
query-chunk wave, which is otherwise ACT(exp)-bound, so the PE has filler
there.  Outputs are stored bf16 (host accumulates in fp32).
Compute dtype bf16 (fp32 PSUM accumulation); bf16 partial outputs.
"""

import numpy as np
import ml_dtypes

import concourse.bass as bass
import concourse.tile as tile
from concourse import bacc, mybir
from concourse.tile import add_dep_helper

D_MODEL = 1024
N_HEADS = 16
D_K = 64
B, L = 4, 2048
TP = 2                  # head groups
GD = D_MODEL // TP      # 512 head-dims per group
P = 128
NQ = 512                # query chunk (one fp32 PSUM bank)
N_MT = GD // P          # 4 M-tiles (head pairs) per group
N_KT = D_MODEL // P     # 8 contraction tiles over model dim
N_TT = L // P           # 16 token tiles
N_QC = L // NQ          # 4 query chunks
BF16 = mybir.dt.bfloat16
F32 = mybir.dt.float32
NPBF16 = ml_dtypes.bfloat16
AF = mybir.ActivationFunctionType
ALU = mybir.AluOpType


def build_nc() -> bass.Bass:
    nc = bacc.Bacc("TRN2", target_bir_lowering=False)

    xqT = nc.dram_tensor("xqT", [D_MODEL, L], BF16, kind="ExternalInput")
    xkT = nc.dram_tensor("xkT", [D_MODEL, L], BF16, kind="ExternalInput")
    xvT = nc.dram_tensor("xvT", [D_MODEL, L], BF16, kind="ExternalInput")
    wqT = nc.dram_tensor("wqT", [D_MODEL, GD], BF16, kind="ExternalInput")
    wkT = nc.dram_tensor("wkT", [D_MODEL, GD], BF16, kind="ExternalInput")
    wvT = nc.dram_tensor("wvT", [D_MODEL, GD], BF16, kind="ExternalInput")
    woT = nc.dram_tensor("woT", [GD, D_MODEL], BF16, kind="ExternalInput")
    bq = nc.dram_tensor("bq", [P, N_MT], F32, kind="ExternalInput")
    bk = nc.dram_tensor("bk", [P, N_MT], F32, kind="ExternalInput")
    bv = nc.dram_tensor("bv", [1, GD], F32, kind="ExternalInput")
    maskc = nc.dram_tensor("maskc", [P, 2 * P], BF16, kind="ExternalInput")
    ident = nc.dram_tensor("ident", [P, P], BF16, kind="ExternalInput")
    out = nc.dram_tensor("out", [L, D_MODEL], BF16, kind="ExternalOutput")

    # DRAM views with the contraction-tile dim split out, for batched DMAs
    xkT3 = xkT.rearrange("(k p) l -> p k l", p=P)
    xqT3 = xqT.rearrange("(k p) l -> p k l", p=P)
    xvT3 = xvT.rearrange("(k p) l -> p k l", p=P)
    wkT3 = wkT.rearrange("(k p) g -> p k g", p=P)
    wqT3 = wqT.rearrange("(k p) g -> p k g", p=P)
    wvT3 = wvT.rearrange("(k p) g -> p k g", p=P)
    woT3 = woT.rearrange("(k p) g -> p k g", p=P)

    with tile.TileContext(nc) as tc:
        with (
            tc.tile_pool(name="const", bufs=1) as const,
            tc.tile_pool(name="xch", bufs=16) as xch_pool,
            tc.tile_pool(name="pt", bufs=4) as pt_pool,
            tc.tile_pool(name="small", bufs=2) as small_pool,
            tc.tile_pool(name="osb", bufs=2) as osb_pool,
            tc.tile_pool(name="acc", bufs=4) as acc_pool,
            tc.tile_pool(name="ps_s", bufs=2, space="PSUM") as ps_s,
            tc.tile_pool(name="ps_av", bufs=2, space="PSUM") as ps_av,
            tc.tile_pool(name="ps_mm", bufs=2, space="PSUM") as ps_mm,
        ):
            scale = float(1.0 / np.sqrt(np.float32(D_K)))

            # memsets first so the warmup matmuls and the reciprocal
            # broadcast constant are ready immediately (DVE queue head)
            w_warm = const.tile([P, NQ], BF16, tag="warm")
            nc.vector.memset(w_warm, 0.125)
            ones64 = const.tile([1, D_K], BF16, tag="ones64")
            nc.vector.memset(ones64, 1.0)

            # PE clock warmup: a few dependency-free matmuls keep the HAM
            # clock gate from seeing a dead PE while the first DMAs land.
            for i in range(6):
                ps_w = ps_mm.tile([P, NQ], F32, tag="mm", name=f"warm{i}")
                nc.tensor.matmul(ps_w, lhsT=w_warm[:, 0:P], rhs=w_warm,
                                 start=True, stop=True)

            # ---- tiny constants first (scalar queue; ACT engine is idle now)
            bk_sb = const.tile([P, N_MT], F32, tag="bk")
            bq_sb = const.tile([P, N_MT], F32, tag="bq")
            bv_sb = const.tile([P, GD], F32, tag="bv")
            mask_sb = const.tile([P, 2, P], BF16, tag="mask")
            nc.scalar.dma_start(out=bk_sb, in_=bk[:, :])
            nc.scalar.dma_start(out=bq_sb, in_=bq[:, :])

            # ---- prologue bulk, sliced per contraction tile and issued in
            # need-order: wk on the scalar queue, wq/wv on the gpsimd queue,
            # first-chunk activations on the sync queue.  Consumers are gated
            # per-slice, so the first projection matmul can start after
            # ~256KB instead of ~2MB.  The DVE queue stays free for the
            # kT/qT epilogues and v bias adds.
            wk_t = [const.tile([P, GD], BF16, tag=f"wk{k}", name=f"wk{k}")
                    for k in range(N_KT)]
            wq_t = [const.tile([P, GD], BF16, tag=f"wq{k}", name=f"wq{k}")
                    for k in range(N_KT)]
            wv_t = [const.tile([P, GD], BF16, tag=f"wv{k}", name=f"wv{k}")
                    for k in range(N_KT)]
            wo_sb = const.tile([P, N_MT, D_MODEL], BF16, tag="wo")
            xk0 = [const.tile([P, NQ], BF16, tag=f"xk0_{k}", name=f"xk0_{k}")
                   for k in range(N_KT)]
            xq0 = [const.tile([P, NQ], BF16, tag=f"xq0_{k}", name=f"xq0_{k}")
                   for k in range(N_KT)]
            xv0 = [const.tile([P, NQ], BF16, tag=f"xv0_{k}", name=f"xv0_{k}")
                   for k in range(N_KT)]

            # Only sync/scalar (HWDGE) and gpsimd (SWDGE) can issue DMAs, and
            # each queue caps at ~150-200GB/s.  Spread the ~6MB critical
            # phase: wk + xv0 on scalar, wq/wv on gpsimd, xk0/xq0
            # interleaved on sync.
            for kt in range(N_KT):
                nc.scalar.dma_start(out=wk_t[kt], in_=wkT3[:, kt, :])
                nc.gpsimd.dma_start(out=wq_t[kt], in_=wqT3[:, kt, :])
                nc.sync.dma_start(out=xk0[kt], in_=xkT3[:, kt, 0:NQ])
                nc.sync.dma_start(out=xq0[kt], in_=xqT3[:, kt, 0:NQ])
            nc.scalar.dma_start(out=mask_sb,
                                in_=maskc.rearrange("p (h n) -> p h n", n=P))
            h_gate = None
            for kt in range(N_KT):
                nc.gpsimd.dma_start(out=wv_t[kt], in_=wvT3[:, kt, :])
                h_gate = nc.scalar.dma_start(out=xv0[kt], in_=xvT3[:, kt, 0:NQ])
            nc.scalar.dma_start(out=bv_sb, in_=bv[:, :].to_broadcast([P, GD]))
            ident_sb = const.tile([P, P], BF16, tag="ident")
            nc.scalar.dma_start(out=ident_sb, in_=ident[:, :])

            # second-chunk k/q activations + second v chunk follow on the sync
            # queue (FIFO behind the critical loads); wo last -- it is only
            # needed once o_proj starts in the final wave.
            xvB = [const.tile([P, N_KT, NQ], BF16, tag=f"xvb{i}", name=f"xvb{i}")
                   for i in range(3)]

            xch_cache = {
                ("k", 0): xk0,
                ("q", 0): xq0,
            }

            def kq_dmas(nm, x_dram, ncz, eng):
                xchs = []
                for kt in range(N_KT):
                    xc = xch_pool.tile([P, NQ], BF16, tag="xch",
                                       name=f"x{nm}{ncz}_{kt}")
                    h = eng.dma_start(
                        out=xc,
                        in_=x_dram[kt * P:(kt + 1) * P, ncz * NQ:(ncz + 1) * NQ],
                    )
                    xchs.append(xc)
                    if nm == "k" and kt == N_KT - 1:
                        pass
                xch_cache[(nm, ncz)] = xchs
                return h

            # chunk-1 loads: k on the scalar queue (idle after the critical
            # weights), q + wo behind the critical x on sync, xvB0 after the
            # weights on gpsimd.
            kq_dmas("k", xkT, 1, nc.scalar)
            kq_dmas("q", xqT, 1, nc.sync)
            nc.sync.dma_start(out=wo_sb, in_=woT3[:, :, :])
            nc.gpsimd.dma_start(out=xvB[0], in_=xvT3[:, :, NQ:2 * NQ])

            # later x loads stay on the gpsimd queue, gated behind the
            # critical phase so they don't steal HBM bandwidth from it.
            h1 = nc.gpsimd.dma_start(out=xvB[1], in_=xvT3[:, :, 2 * NQ:3 * NQ])
            add_dep_helper(h1.ins, h_gate.ins, sync=True,
                           reason="late xv chunks wait for critical phase")

            def v_proj_tile(tt):
                ps = ps_mm.tile([P, GD], F32, tag="mm", name=f"psv{tt}")
                for kt in range(N_KT):
                    if tt < 4:
                        xs = xv0[kt][:, (tt % 4) * P:(tt % 4 + 1) * P]
                    else:
                        xs = xvB[tt // 4 - 1][:, kt, (tt % 4) * P:(tt % 4 + 1) * P]
                    nc.tensor.matmul(
                        ps,
                        lhsT=xs,
                        rhs=wv_t[kt],
                        start=(kt == 0),
                        stop=(kt == N_KT - 1),
                    )
                nc.vector.tensor_tensor(
                    out=vA[tt][:, :, VW - D_K:VW],
                    in0=ps.rearrange("p (h d) -> p h d", d=D_K),
                    in1=bv_sb.rearrange("p (h d) -> p h d", d=D_K),
                    op=ALU.add,
                )
                nc.vector.memset(vA[tt][:, :, 1:VW - D_K], 0.0)
                nc.vector.memset(vA[tt][:, :, 0:1], 1.0)

            def kq_part(w_t, b_sb, dsts, sc, nm, ncz, mt):
                xchs = xch_cache[(nm, ncz)]
                ps = ps_mm.tile([P, NQ], F32, tag="mm", name=f"ps{nm}{ncz}{mt}")
                for kt in range(N_KT):
                    nc.tensor.matmul(
                        ps,
                        lhsT=w_t[kt][:, mt * P:(mt + 1) * P],
                        rhs=xchs[kt],
                        start=(kt == 0),
                        stop=(kt == N_KT - 1),
                    )
                nc.vector.tensor_scalar(
                    out=dsts[mt][ncz],
                    in0=ps,
                    scalar1=b_sb[:, mt:mt + 1],
                    scalar2=sc,
                    op0=ALU.add,
                    op1=ALU.mult,
                )

            # per-(mt, chunk) tiles so consumers unblock as soon as possible
            qTt = [[const.tile([P, NQ], BF16, tag=f"qT{mt}_{ncz}", name=f"qT{mt}_{ncz}")
                    for ncz in range(N_QC)] for mt in range(N_MT)]
            kTt = [[const.tile([P, NQ], BF16, tag=f"kT{mt}_{ncz}", name=f"kT{mt}_{ncz}")
                    for ncz in range(N_QC)] for mt in range(N_MT)]
            # 128 cols per head: [ones, 63 zeros, 64 V-dims] so the AV output
            # puts the denominator at partition 0 and V rows at partition 64
            VW = 128
            vA = [const.tile([P, 2 * N_MT, VW], BF16, tag=f"v{tt}", name=f"v{tt}")
                  for tt in range(N_TT)]
            aoTq = [[const.tile([P, NQ], BF16, tag=f"ao{mt}_{qc}", name=f"ao{mt}_{qc}")
                     for qc in range(N_QC)] for mt in range(N_MT)]

            # ---- attention, software-pipelined: the AV pair for a key-block
            # is emitted two blocks after its scores/exp, so the PE (strict
            # FIFO queue) has scores work while ACT runs exp and never stalls
            # on a single exp's latency.
            pipe = {"q": []}

            def normalize(mt, qc, acc):
                for h2 in range(2):
                    rec1 = small_pool.tile([1, NQ], F32, tag="rec1",
                                           name=f"rec1{mt}_{qc}_{h2}")
                    nc.vector.reciprocal_approx_fast(
                        out=rec1, in_=acc[h2][0:1, :])
                    recb = small_pool.tile([1, NQ], BF16, tag="recb",
                                           name=f"recb{mt}_{qc}_{h2}")
                    nc.vector.tensor_copy(out=recb, in_=rec1)
                    # broadcast to partitions 64..127 via a K=1 outer product
                    # on the PE (partition-broadcast DMA is broken on HW, and
                    # a DRAM bounce costs ~3us of latency + 2 DMAs)
                    bc = ps_mm.tile([P, NQ], F32, tag="mm",
                                    name=f"bc{mt}_{qc}_{h2}")
                    nc.tensor.matmul(bc[VW - D_K:VW, :], lhsT=ones64[0:1, :],
                                     rhs=recb, start=True, stop=True)
                    nc.vector.tensor_tensor(
                        out=aoTq[mt][qc][h2 * D_K:(h2 + 1) * D_K, :],
                        in0=acc[h2][VW - D_K:VW, :],
                        in1=bc[VW - D_K:VW, :],
                        op=ALU.mult,
                    )

            def emit_av(st):
                mt, qc, kb, p3, av, acc = st
                nkb = 4 * qc + 4
                t = P * (kb - 4 * qc)
                grp_start = (kb % 8 == 0)
                grp_stop = (kb % 8 == 7) or (kb == nkb - 1)
                for h2 in range(2):
                    nc.tensor.matmul(
                        av[h2][:, max(t, 0):NQ],
                        lhsT=vA[kb][:, 2 * mt + h2, :],
                        rhs=p3[:, h2, max(t, 0):NQ],
                        start=grp_start,
                        stop=grp_stop,
                    )
                if grp_stop:  # evict group into SBUF accumulator
                    for h2 in range(2):
                        if kb < 8:
                            nc.vector.tensor_copy(out=acc[h2], in_=av[h2])
                        else:
                            nc.vector.tensor_tensor(
                                out=acc[h2], in0=acc[h2], in1=av[h2],
                                op=ALU.add,
                            )
                if kb == nkb - 1:
                    # pair complete: normalize here so the DVE queue sees the
                    # eviction before the ops that consume the accumulator.
                    normalize(mt, qc, acc)

            def attention_pair(mt, qc, interleave=None, warm=False):
                acc = [acc_pool.tile([VW, NQ], F32, tag="acc",
                                     name=f"acc{mt}_{qc}_{i}") for i in range(2)]
                av = [None, None]
                nkb = 4 * qc + 4
                for kb in range(nkb):
                    if kb % 8 == 0:
                        av = [ps_av.tile([VW, NQ], F32, tag="av",
                                         name=f"av{mt}_{qc}_{kb}_{i}")
                              for i in range(2)]
                    t = P * (kb - 4 * qc)  # <0 for full blocks
                    s_ps = ps_s.tile([P, 2 * NQ], F32, tag="s",
                                     name=f"s{mt}_{qc}_{kb}")
                    s3 = s_ps.rearrange("p (h n) -> p h n", n=NQ)
                    for h2 in range(2):
                        nc.tensor.matmul(
                            s3[:, h2, max(t, 0):NQ],
                            lhsT=kTt[mt][kb // 4][h2 * D_K:(h2 + 1) * D_K,
                                                 (kb % 4) * P:(kb % 4 + 1) * P],
                            rhs=qTt[mt][qc][h2 * D_K:(h2 + 1) * D_K,
                                            max(t, 0):NQ],
                            start=True,
                            stop=True,
                        )
                    pt = pt_pool.tile([P, 2 * NQ], BF16, tag="pt",
                                      name=f"pt{mt}_{qc}_{kb}")
                    p3 = pt.rearrange("p (h n) -> p h n", n=NQ)
                    if t <= 0:
                        nc.scalar.activation(out=pt, in_=s_ps, func=AF.Exp)
                    else:
                        nc.scalar.activation(out=p3[:, :, t:NQ],
                                             in_=s3[:, :, t:NQ], func=AF.Exp)
                    if t >= 0:  # diagonal sub-block: fused triangular mask
                        nc.vector.tensor_tensor(
                            out=p3[:, :, t:t + P],
                            in0=p3[:, :, t:t + P],
                            in1=mask_sb,
                            op=ALU.mult,
                        )
                    pipe["q"].append((mt, qc, kb, p3, av, acc))
                    if len(pipe["q"]) > 2:
                        emit_av(pipe["q"].pop(0))
                    if interleave is not None:
                        interleave(kb)

            def flush_av():
                while pipe["q"]:
                    emit_av(pipe["q"].pop(0))

            def o_proj_piece(qc, j, dc):
                lt = 4 * qc + j
                ps = ps_mm.tile([P, NQ], F32, tag="mm", name=f"po{lt}_{dc}")
                for kt in range(N_MT):
                    nc.tensor.matmul(
                        ps,
                        lhsT=aoTq[kt][qc][:, j * P:(j + 1) * P],
                        rhs=wo_sb[:, kt, dc * NQ:(dc + 1) * NQ],
                        start=(kt == 0),
                        stop=(kt == N_MT - 1),
                    )
                ot = osb_pool.tile([P, NQ], BF16, tag="ot", name=f"ot{lt}_{dc}")
                nc.vector.tensor_copy(out=ot, in_=ps)
                nc.sync.dma_start(
                    out=out[lt * P:(lt + 1) * P, dc * NQ:(dc + 1) * NQ],
                    in_=ot,
                )

            def park_piece(j, dc):
                # o_proj for the last wave, head-pairs 0..2 only; the kt=3
                # contribution joins after normalize(3,3) in the tail.
                pso = ps_mm.tile([P, NQ], F32, tag="mm", name=f"pop{j}_{dc}")
                for kt in range(3):
                    nc.tensor.matmul(
                        pso,
                        lhsT=aoTq[kt][3][:, j * P:(j + 1) * P],
                        rhs=wo_sb[:, kt, dc * NQ:(dc + 1) * NQ],
                        start=(kt == 0),
                        stop=(kt == 2),
                    )
                nc.vector.tensor_copy(out=xk0[2 * j + dc], in_=pso)

            # ---- emission schedule.  Prologue: six chunk-0 chains run
            # kt-MAJOR (their matmuls interleaved slice-by-slice), borrowing
            # the idle ps_s/ps_av banks, so every arriving x-slice unblocks
            # six matmuls instead of one -- the strict-FIFO PE queue never
            # waits on one chain's slowest slice.  v_proj tiles (whose
            # wv/xv0 data lands last) interleave into pair (0,0).
            six = [
                ("k", 0, ps_s, "s"), ("q", 0, ps_s, "s"),
                ("k", 1, ps_av, "av"), ("q", 1, ps_av, "av"),
                ("k", 2, ps_mm, "mm"), ("k", 3, ps_mm, "mm"),
            ]
            pss6 = [pool.tile([P, NQ], F32, tag=tag, name=f"pc{nm}{mt}")
                    for nm, mt, pool, tag in six]
            for kt in range(N_KT):
                for i, (nm, mt, pool, tag) in enumerate(six):
                    w_t = wk_t if nm == "k" else wq_t
                    nc.tensor.matmul(
                        pss6[i],
                        lhsT=w_t[kt][:, mt * P:(mt + 1) * P],
                        rhs=xch_cache[(nm, 0)][kt],
                        start=(kt == 0),
                        stop=(kt == N_KT - 1),
                    )
            for i, (nm, mt, pool, tag) in enumerate(six):
                b_sb, dsts, sc = ((bk_sb, kTt, 1.0) if nm == "k"
                                  else (bq_sb, qTt, scale))
                nc.vector.tensor_scalar(
                    out=dsts[mt][0], in0=pss6[i],
                    scalar1=b_sb[:, mt:mt + 1], scalar2=sc,
                    op0=ALU.add, op1=ALU.mult,
                )
            kq_part(wq_t, bq_sb, qTt, scale, "q", 0, 2)
            kq_part(wq_t, bq_sb, qTt, scale, "q", 0, 3)

            def prologue_fill(kb):
                v_proj_tile(kb)

            def wave3_fill(mt):
                # During the exp-bound last wave, interleave all deferred
                # o_proj work: pieces of chunk mt during pair (mt,3) for
                # mt<3, the kt0..2 partials of chunk 3 during pair (3,3).
                def fill(kb):
                    if kb % 2 == 1:
                        i = kb // 2
                        if mt < 3:
                            o_proj_piece(mt, i // 2, i % 2)
                        else:
                            park_piece(i // 2, i % 2)
                return fill

            for qc in range(N_QC):
                if 1 <= qc < 3:
                    kq_dmas("k", xkT, qc + 1, nc.gpsimd)
                    kq_dmas("q", xqT, qc + 1, nc.gpsimd)
                    if qc == 1:
                        nc.gpsimd.dma_start(out=xvB[2],
                                            in_=xvT3[:, :, 3 * NQ:4 * NQ])
                for mt in range(N_MT):
                    if qc == 0 and mt == 0:
                        attention_pair(0, 0, interleave=prologue_fill,
                                       warm=True)
                    elif qc == 3:
                        attention_pair(mt, 3, interleave=wave3_fill(mt))
                    else:
                        attention_pair(mt, qc, warm=(qc == 0))
                    if qc < 3:
                        nz = qc + 1
                        kq_part(wk_t, bk_sb, kTt, 1.0, "k", nz, mt)
                        v_proj_tile(4 * nz + mt)
                        kq_part(wq_t, bq_sb, qTt, scale, "q", nz, mt)
                flush_av()
            # ---- tail: the kt=3 contribution of the last wave's o_proj
            # joins the parked kt0..2 partial via an identity-matmul inject
            # (PSUM accumulate), so the only post-PE work per piece is one
            # copy -- alternated DVE / ACT, both idle here.
            for j in range(4):
                for dc in range(2):
                    lt = 12 + j
                    pool = ps_av if (2 * j + dc) % 2 else ps_mm
                    ps2 = pool.tile([P, NQ], F32,
                                    tag="av" if (2 * j + dc) % 2 else "mm",
                                    name=f"pof{j}_{dc}")
                    nc.tensor.matmul(
                        ps2,
                        lhsT=aoTq[3][3][:, j * P:(j + 1) * P],
                        rhs=wo_sb[:, 3, dc * NQ:(dc + 1) * NQ],
                        start=True,
                        stop=False,
                    )
                    nc.tensor.matmul(
                        ps2,
                        lhsT=ident_sb,
                        rhs=xk0[2 * j + dc],
                        start=False,
                        stop=True,
                    )
                    ot = osb_pool.tile([P, NQ], BF16, tag="ot",
                                       name=f"ot{lt}_{dc}")
                    if dc == 0:
                        nc.vector.tensor_copy(out=ot, in_=ps2)
                    else:
                        nc.scalar.copy(out=ot, in_=ps2)
                    nc.sync.dma_start(
                        out=out[lt * P:(lt + 1) * P, dc * NQ:(dc + 1) * NQ],
                        in_=ot,
                    )
    nc.finalize()
    return nc


def make_in_maps(Q, K, V, Wq, bq, Wk, bk, Wv, bv, Wo, bo, attn_mask=None):
    """Build the 8 per-core input maps from full (unsharded) inputs."""
    Q = np.asarray(Q, np.float32)
    K = np.asarray(K, np.float32)
    V = np.asarray(V, np.float32)
    Wq = np.asarray(Wq, np.float32)
    Wk = np.asarray(Wk, np.float32)
    Wv = np.asarray(Wv, np.float32)
    Wo = np.asarray(Wo, np.float32)
    bq = np.asarray(bq, np.float32)
    bk = np.asarray(bk, np.float32)
    bv = np.asarray(bv, np.float32)

    i_idx = np.arange(P)[:, None]
    j_idx = np.arange(P)[None, :]
    tri = (i_idx <= j_idx).astype(NPBF16)
    maskc = np.concatenate([tri, tri], axis=1)  # duplicated for the 2 heads
    ident = np.eye(P, dtype=NPBF16)

    xT = {}
    for b in range(B):
        xT[b] = tuple(
            np.ascontiguousarray(X[b].T).astype(NPBF16) for X in (Q, K, V)
        )
    grp = {}
    for g in range(TP):
        sl = slice(g * GD, (g + 1) * GD)
        grp[g] = dict(
            wqT=np.ascontiguousarray(Wq[sl, :].T).astype(NPBF16),
            wkT=np.ascontiguousarray(Wk[sl, :].T).astype(NPBF16),
            wvT=np.ascontiguousarray(Wv[sl, :].T).astype(NPBF16),
            woT=np.ascontiguousarray(Wo[:, sl].T).astype(NPBF16),
            bq=np.ascontiguousarray(bq[sl].reshape(N_MT, P).T).astype(np.float32),
            bk=np.ascontiguousarray(bk[sl].reshape(N_MT, P).T).astype(np.float32),
            bv=np.ascontiguousarray(bv[sl].reshape(1, GD)).astype(np.float32),
        )
    in_maps = []
    for c in range(2 * B):
        b, g = c // 2, c % 2
        m = dict(grp[g])
        m["xqT"], m["xkT"], m["xvT"] = xT[b]
        m["maskc"] = maskc
        m["ident"] = ident
        in_maps.append(m)
    return in_maps


def assemble_output(results, bo):
    bo = np.asarray(bo, np.float32)
    out = np.empty((B, L, D_MODEL), np.float32)
    for b in range(B):
        out[b] = (results[2 * b]["out"].astype(np.float32)
                  + results[2 * b + 1]["out"].astype(np.float32) + bo)
    return out


_NC_CACHE = None


def kernel(**inputs) -> np.ndarray:
    global _NC_CACHE
    from concourse.bass_utils import run_bass_kernel_spmd

    if _NC_CACHE is None:
        _NC_CACHE = build_nc()
    in_maps = make_in_maps(**inputs)
    res = run_bass_kernel_spmd(_NC_CACHE, in_maps, core_ids=list(range(2 * B)))
    return assemble_output(res.results, inputs["bo"])


# revision 49
# speedup vs baseline: 1.0119x; 1.0119x over previous
"""Multi-head causal attention (QKV proj + attention + O proj) on 8 TRN2 cores.

Sharding: data-parallel over batch (4) x tensor-parallel over heads (2 groups
of 8 heads).  Core c handles batch c//2, head-group c%2.  Each core computes
its group's partial o_proj output; the host sums the two partials per batch.

Layout strategy (all activations arrive pre-transposed from the host, so the
kernel never transposes on-device):
  - qT, kT per head-pair M-tile: (128 head-dims, L) from  W.T-slice @ X.T
  - v natural (tokens, head-dims) with a fused ones-column for the softmax
    denominator: av_psum = v_aug.T @ P.T gives (65, Nq) where row 64 is the
    per-query sum of probabilities.
  - scores are computed transposed (keys on partitions, queries free); the two
    heads of an M-tile run as concurrent PE row-tiles (K=64 at row 0 / 64).
  - AV emission is software-pipelined one key-block behind scores/exp so the
    strict-FIFO PE queue never stalls waiting on the ACT-engine exp.
  - causal diagonal blocks: exp first, then one fused tensor_tensor multiply
    (both heads at once) against a host-duplicated triangular mask.
  - softmax denominators: fast-approx reciprocal on the 1-partition row (bf16),
    then a K=1 PE matmul (ones outer product) broadcasts it to partitions
    64..127 of a PSUM bank -- no DRAM bounce.
Schedule: the prologue DMAs are sliced per contraction tile and spread over
three engine queues in need-order (vector: weights, sync: x chunks, scalar:
tiny constants) so projection chains start within ~3us; later x chunks are
gated behind the critical phase.  All o_proj work is deferred to the last
query-chunk wave, which is otherwise ACT(exp)-bound, so the PE has filler
there.  Outputs are stored bf16 (host accumulates in fp32).
Compute dtype bf16 (fp32 PSUM accumulation); bf16 partial outputs.
"""

import numpy as np
import ml_dtypes

import concourse.bass as bass
import concourse.tile as tile
from concourse import bacc, mybir
from concourse.tile import add_dep_helper

D_MODEL = 1024
N_HEADS = 16
D_K = 64
B, L = 4, 2048
TP = 2                  # head groups
GD = D_MODEL // TP      # 512 head-dims per group
P = 128
NQ = 512                # query chunk (one fp32 PSUM bank)
N_MT = GD // P          # 4 M-tiles (head pairs) per group
N_KT = D_MODEL // P     # 8 contraction tiles over model dim
N_TT = L // P           # 16 token tiles
N_QC = L // NQ          # 4 query chunks
BF16 = mybir.dt.bfloat16
F32 = mybir.dt.float32
NPBF16 = ml_dtypes.bfloat16
AF = mybir.ActivationFunctionType
ALU = mybir.AluOpType


def build_nc() -> bass.Bass:
    nc = bacc.Bacc("TRN2", target_bir_lowering=False)

    # All bulk inputs arrive in partition-major layouts with >=8KB of
    # contiguous data per partition line -- 1KB lines measure ~50-90GB/s
    # per DMA queue, 8KB lines ~250GB/s.
    xqT = nc.dram_tensor("xqT", [P, N_QC, N_KT, NQ], BF16, kind="ExternalInput")
    xkT = nc.dram_tensor("xkT", [P, N_QC, N_KT, NQ], BF16, kind="ExternalInput")
    xvT = nc.dram_tensor("xvT", [P, N_QC, N_KT, NQ], BF16, kind="ExternalInput")
    wqT = nc.dram_tensor("wqT", [P, N_KT, GD], BF16, kind="ExternalInput")
    wkT = nc.dram_tensor("wkT", [P, N_KT, GD], BF16, kind="ExternalInput")
    wvT = nc.dram_tensor("wvT", [P, N_KT, GD], BF16, kind="ExternalInput")
    woT = nc.dram_tensor("woT", [P, N_MT, D_MODEL], BF16, kind="ExternalInput")
    bq = nc.dram_tensor("bq", [P, N_MT], F32, kind="ExternalInput")
    bk = nc.dram_tensor("bk", [P, N_MT], F32, kind="ExternalInput")
    bv = nc.dram_tensor("bv", [1, GD], F32, kind="ExternalInput")
    maskc = nc.dram_tensor("maskc", [P, 2 * P], BF16, kind="ExternalInput")
    ident = nc.dram_tensor("ident", [P, P], BF16, kind="ExternalInput")
    out = nc.dram_tensor("out", [L, D_MODEL], BF16, kind="ExternalOutput")

    with tile.TileContext(nc) as tc:
        with (
            tc.tile_pool(name="const", bufs=1) as const,
            tc.tile_pool(name="xch", bufs=3) as xch_pool,
            tc.tile_pool(name="qT", bufs=9) as qT_pool,
            tc.tile_pool(name="pt", bufs=4) as pt_pool,
            tc.tile_pool(name="small", bufs=2) as small_pool,
            tc.tile_pool(name="osb", bufs=2) as osb_pool,
            tc.tile_pool(name="acc", bufs=4) as acc_pool,
            tc.tile_pool(name="ps_s", bufs=2, space="PSUM") as ps_s,
            tc.tile_pool(name="ps_av", bufs=2, space="PSUM") as ps_av,
            tc.tile_pool(name="ps_mm", bufs=2, space="PSUM") as ps_mm,
        ):
            scale = float(1.0 / np.sqrt(np.float32(D_K)))

            # memsets first so the warmup matmuls and the reciprocal
            # broadcast constant are ready immediately (DVE queue head)
            w_warm = const.tile([P, NQ], BF16, tag="warm")
            nc.vector.memset(w_warm, 0.125)
            ones64 = const.tile([1, D_K], BF16, tag="ones64")
            nc.vector.memset(ones64, 1.0)

            # PE clock warmup: a few dependency-free matmuls keep the HAM
            # clock gate from seeing a dead PE while the first DMAs land.
            for i in range(6):
                ps_w = ps_mm.tile([P, NQ], F32, tag="mm", name=f"warm{i}")
                nc.tensor.matmul(ps_w, lhsT=w_warm[:, 0:P], rhs=w_warm,
                                 start=True, stop=True)

            # ---- tiny constants first (scalar queue; ACT engine is idle now)
            bk_sb = const.tile([P, N_MT], F32, tag="bk")
            bq_sb = const.tile([P, N_MT], F32, tag="bq")
            bv_sb = const.tile([P, GD], F32, tag="bv")
            mask_sb = const.tile([P, 2, P], BF16, tag="mask")
            nc.scalar.dma_start(out=bk_sb, in_=bk[:, :])
            nc.scalar.dma_start(out=bq_sb, in_=bq[:, :])

            # ---- prologue bulk: one whole-tensor DMA each (8KB partition
            # lines: 1KB lines measure ~50-90GB/s per queue, 8KB ~250GB/s),
            # spread across the three queues in need-order.  The DVE queue
            # stays free for the kT/qT epilogues and v bias adds.
            wk_sb = const.tile([P, N_KT, GD], BF16, tag="wk")
            wq_sb = const.tile([P, N_KT, GD], BF16, tag="wq")
            wv_sb = const.tile([P, N_KT, GD], BF16, tag="wv")
            wo_sb = const.tile([P, N_MT, D_MODEL], BF16, tag="wo")
            xk0 = const.tile([P, N_KT, NQ], BF16, tag="xk0")
            xq0 = const.tile([P, N_KT, NQ], BF16, tag="xq0")
            xv0 = const.tile([P, N_KT, NQ], BF16, tag="xv0")

            nc.scalar.dma_start(out=wk_sb, in_=wkT[:, :, :])
            nc.gpsimd.dma_start(out=wq_sb, in_=wqT[:, :, :])
            nc.sync.dma_start(out=xk0, in_=xkT[:, 0, :, :])
            nc.sync.dma_start(out=xq0, in_=xqT[:, 0, :, :])
            nc.scalar.dma_start(out=mask_sb,
                                in_=maskc.rearrange("p (h n) -> p h n", n=P))
            nc.scalar.dma_start(out=wv_sb, in_=wvT[:, :, :])
            h_gate = nc.sync.dma_start(out=xv0, in_=xvT[:, 0, :, :])
            nc.scalar.dma_start(out=bv_sb, in_=bv[:, :].to_broadcast([P, GD]))
            ident_sb = const.tile([P, P], BF16, tag="ident")
            nc.scalar.dma_start(out=ident_sb, in_=ident[:, :])
            wk_t = [wk_sb[:, kt, :] for kt in range(N_KT)]
            wq_t = [wq_sb[:, kt, :] for kt in range(N_KT)]
            wv_t = [wv_sb[:, kt, :] for kt in range(N_KT)]

            xvB = [const.tile([P, N_KT, NQ], BF16, tag=f"xvb{i}", name=f"xvb{i}")
                   for i in range(3)]

            xch_cache = {
                ("k", 0): [xk0[:, kt, :] for kt in range(N_KT)],
                ("q", 0): [xq0[:, kt, :] for kt in range(N_KT)],
            }

            def kq_dmas(nm, x_dram, ncz, eng):
                xc = xch_pool.tile([P, N_KT, NQ], BF16, tag="xch",
                                   name=f"x{nm}{ncz}")
                h = eng.dma_start(out=xc, in_=x_dram[:, ncz, :, :])
                xch_cache[(nm, ncz)] = [xc[:, kt, :] for kt in range(N_KT)]
                return h

            # chunk-1 loads: k on the scalar queue (idle after the critical
            # weights), q + wo behind the critical x on sync, xvB0 on gpsimd.
            kq_dmas("k", xkT, 1, nc.scalar)
            kq_dmas("q", xqT, 1, nc.sync)
            nc.sync.dma_start(out=wo_sb, in_=woT[:, :, :])
            nc.gpsimd.dma_start(out=xvB[0], in_=xvT[:, 1, :, :])

            # later x loads stay on the gpsimd queue, gated behind the
            # critical phase so they don't steal HBM bandwidth from it.
            h1 = nc.gpsimd.dma_start(out=xvB[1], in_=xvT[:, 2, :, :])
            add_dep_helper(h1.ins, h_gate.ins, sync=True,
                           reason="late xv chunks wait for critical phase")

            def v_proj_tile(tt):
                ps = ps_mm.tile([P, GD], F32, tag="mm", name=f"psv{tt}")
                for kt in range(N_KT):
                    if tt < 4:
                        xs = xv0[:, kt, (tt % 4) * P:(tt % 4 + 1) * P]
                    else:
                        xs = xvB[tt // 4 - 1][:, kt, (tt % 4) * P:(tt % 4 + 1) * P]
                    nc.tensor.matmul(
                        ps,
                        lhsT=xs,
                        rhs=wv_t[kt],
                        start=(kt == 0),
                        stop=(kt == N_KT - 1),
                    )
                nc.vector.tensor_tensor(
                    out=vA[tt][:, :, VW - D_K:VW],
                    in0=ps.rearrange("p (h d) -> p h d", d=D_K),
                    in1=bv_sb.rearrange("p (h d) -> p h d", d=D_K),
                    op=ALU.add,
                )
                nc.vector.memset(vA[tt][:, :, 1:VW - D_K], 0.0)
                nc.vector.memset(vA[tt][:, :, 0:1], 1.0)

            def kq_part(w_t, b_sb, dsts, sc, nm, ncz, mt):
                xchs = xch_cache[(nm, ncz)]
                if nm == "q":
                    dsts[mt][ncz] = qT_pool.tile([P, NQ], BF16, tag="qT",
                                                 name=f"qT{mt}_{ncz}")
                ps = ps_mm.tile([P, NQ], F32, tag="mm", name=f"ps{nm}{ncz}{mt}")
                for kt in range(N_KT):
                    nc.tensor.matmul(
                        ps,
                        lhsT=w_t[kt][:, mt * P:(mt + 1) * P],
                        rhs=xchs[kt],
                        start=(kt == 0),
                        stop=(kt == N_KT - 1),
                    )
                nc.vector.tensor_scalar(
                    out=dsts[mt][ncz],
                    in0=ps,
                    scalar1=b_sb[:, mt:mt + 1],
                    scalar2=sc,
                    op0=ALU.add,
                    op1=ALU.mult,
                )

            # per-(mt, chunk) tiles so consumers unblock as soon as possible
            qTt = [[None for _ in range(N_QC)] for _ in range(N_MT)]
            kTt = [[const.tile([P, NQ], BF16, tag=f"kT{mt}_{ncz}", name=f"kT{mt}_{ncz}")
                    for ncz in range(N_QC)] for mt in range(N_MT)]
            # 128 cols per head: [ones, 63 zeros, 64 V-dims] so the AV output
            # puts the denominator at partition 0 and V rows at partition 64
            VW = 128
            vA = [const.tile([P, 2 * N_MT, VW], BF16, tag=f"v{tt}", name=f"v{tt}")
                  for tt in range(N_TT)]
            aoTq = [[const.tile([P, NQ], BF16, tag=f"ao{mt}_{qc}", name=f"ao{mt}_{qc}")
                     for qc in range(N_QC)] for mt in range(N_MT)]

            # ---- attention, software-pipelined: the AV pair for a key-block
            # is emitted two blocks after its scores/exp, so the PE (strict
            # FIFO queue) has scores work while ACT runs exp and never stalls
            # on a single exp's latency.
            pipe = {"q": []}

            def normalize(mt, qc, acc):
                for h2 in range(2):
                    rec1 = small_pool.tile([1, NQ], F32, tag="rec1",
                                           name=f"rec1{mt}_{qc}_{h2}")
                    nc.vector.reciprocal_approx_fast(
                        out=rec1, in_=acc[h2][0:1, :])
                    recb = small_pool.tile([1, NQ], BF16, tag="recb",
                                           name=f"recb{mt}_{qc}_{h2}")
                    nc.vector.tensor_copy(out=recb, in_=rec1)
                    # broadcast to partitions 64..127 via a K=1 outer product
                    # on the PE (partition-broadcast DMA is broken on HW, and
                    # a DRAM bounce costs ~3us of latency + 2 DMAs)
                    bc = ps_mm.tile([P, NQ], F32, tag="mm",
                                    name=f"bc{mt}_{qc}_{h2}")
                    nc.tensor.matmul(bc[VW - D_K:VW, :], lhsT=ones64[0:1, :],
                                     rhs=recb, start=True, stop=True)
                    nc.vector.tensor_tensor(
                        out=aoTq[mt][qc][h2 * D_K:(h2 + 1) * D_K, :],
                        in0=acc[h2][VW - D_K:VW, :],
                        in1=bc[VW - D_K:VW, :],
                        op=ALU.mult,
                    )

            def emit_av(st):
                mt, qc, kb, p3, av, acc = st
                nkb = 4 * qc + 4
                t = P * (kb - 4 * qc)
                grp_start = (kb % 8 == 0)
                grp_stop = (kb % 8 == 7) or (kb == nkb - 1)
                for h2 in range(2):
                    nc.tensor.matmul(
                        av[h2][:, max(t, 0):NQ],
                        lhsT=vA[kb][:, 2 * mt + h2, :],
                        rhs=p3[:, h2, max(t, 0):NQ],
                        start=grp_start,
                        stop=grp_stop,
                    )
                if grp_stop:  # evict group into SBUF accumulator
                    for h2 in range(2):
                        if kb < 8:
                            nc.vector.tensor_copy(out=acc[h2], in_=av[h2])
                        else:
                            nc.vector.tensor_tensor(
                                out=acc[h2], in0=acc[h2], in1=av[h2],
                                op=ALU.add,
                            )
                if kb == nkb - 1:
                    # pair complete: normalize here so the DVE queue sees the
                    # eviction before the ops that consume the accumulator.
                    normalize(mt, qc, acc)

            def attention_pair(mt, qc, interleave=None, warm=False):
                acc = [acc_pool.tile([VW, NQ], F32, tag="acc",
                                     name=f"acc{mt}_{qc}_{i}") for i in range(2)]
                av = [None, None]
                nkb = 4 * qc + 4
                for kb in range(nkb):
                    if kb % 8 == 0:
                        av = [ps_av.tile([VW, NQ], F32, tag="av",
                                         name=f"av{mt}_{qc}_{kb}_{i}")
                              for i in range(2)]
                    t = P * (kb - 4 * qc)  # <0 for full blocks
                    s_ps = ps_s.tile([P, 2 * NQ], F32, tag="s",
                                     name=f"s{mt}_{qc}_{kb}")
                    s3 = s_ps.rearrange("p (h n) -> p h n", n=NQ)
                    for h2 in range(2):
                        nc.tensor.matmul(
                            s3[:, h2, max(t, 0):NQ],
                            lhsT=kTt[mt][kb // 4][h2 * D_K:(h2 + 1) * D_K,
                                                 (kb % 4) * P:(kb % 4 + 1) * P],
                            rhs=qTt[mt][qc][h2 * D_K:(h2 + 1) * D_K,
                                            max(t, 0):NQ],
                            start=True,
                            stop=True,
                        )
                    pt = pt_pool.tile([P, 2 * NQ], BF16, tag="pt",
                                      name=f"pt{mt}_{qc}_{kb}")
                    p3 = pt.rearrange("p (h n) -> p h n", n=NQ)
                    if t <= 0:
                        nc.scalar.activation(out=pt, in_=s_ps, func=AF.Exp)
                    else:
                        nc.scalar.activation(out=p3[:, :, t:NQ],
                                             in_=s3[:, :, t:NQ], func=AF.Exp)
                    if t >= 0:  # diagonal sub-block: fused triangular mask
                        nc.vector.tensor_tensor(
                            out=p3[:, :, t:t + P],
                            in0=p3[:, :, t:t + P],
                            in1=mask_sb,
                            op=ALU.mult,
                        )
                    pipe["q"].append((mt, qc, kb, p3, av, acc))
                    if len(pipe["q"]) > 2:
                        emit_av(pipe["q"].pop(0))
                    if interleave is not None:
                        interleave(kb)

            def flush_av():
                while pipe["q"]:
                    emit_av(pipe["q"].pop(0))

            def o_proj_piece(qc, j, dc):
                lt = 4 * qc + j
                ps = ps_mm.tile([P, NQ], F32, tag="mm", name=f"po{lt}_{dc}")
                for kt in range(N_MT):
                    nc.tensor.matmul(
                        ps,
                        lhsT=aoTq[kt][qc][:, j * P:(j + 1) * P],
                        rhs=wo_sb[:, kt, dc * NQ:(dc + 1) * NQ],
                        start=(kt == 0),
                        stop=(kt == N_MT - 1),
                    )
                ot = osb_pool.tile([P, NQ], BF16, tag="ot", name=f"ot{lt}_{dc}")
                nc.vector.tensor_copy(out=ot, in_=ps)
                nc.sync.dma_start(
                    out=out[lt * P:(lt + 1) * P, dc * NQ:(dc + 1) * NQ],
                    in_=ot,
                )

            def park_piece(j, dc):
                # o_proj for the last wave, head-pairs 0..2 only; the kt=3
                # contribution joins after normalize(3,3) in the tail.
                pso = ps_mm.tile([P, NQ], F32, tag="mm", name=f"pop{j}_{dc}")
                for kt in range(3):
                    nc.tensor.matmul(
                        pso,
                        lhsT=aoTq[kt][3][:, j * P:(j + 1) * P],
                        rhs=wo_sb[:, kt, dc * NQ:(dc + 1) * NQ],
                        start=(kt == 0),
                        stop=(kt == 2),
                    )
                nc.vector.tensor_copy(out=xk0[:, 2 * j + dc, :], in_=pso)

            # ---- emission schedule.  Prologue: six chunk-0 chains run
            # kt-MAJOR (their matmuls interleaved slice-by-slice), borrowing
            # the idle ps_s/ps_av banks, so every arriving x-slice unblocks
            # six matmuls instead of one -- the strict-FIFO PE queue never
            # waits on one chain's slowest slice.  v_proj tiles (whose
            # wv/xv0 data lands last) interleave into pair (0,0).
            six = [
                ("k", 0, ps_s, "s"), ("q", 0, ps_s, "s"),
                ("k", 1, ps_av, "av"), ("q", 1, ps_av, "av"),
                ("k", 2, ps_mm, "mm"), ("k", 3, ps_mm, "mm"),
            ]
            pss6 = [pool.tile([P, NQ], F32, tag=tag, name=f"pc{nm}{mt}")
                    for nm, mt, pool, tag in six]
            for kt in range(N_KT):
                for i, (nm, mt, pool, tag) in enumerate(six):
                    w_t = wk_t if nm == "k" else wq_t
                    nc.tensor.matmul(
                        pss6[i],
                        lhsT=w_t[kt][:, mt * P:(mt + 1) * P],
                        rhs=xch_cache[(nm, 0)][kt],
                        start=(kt == 0),
                        stop=(kt == N_KT - 1),
                    )
            for i, (nm, mt, pool, tag) in enumerate(six):
                b_sb, dsts, sc = ((bk_sb, kTt, 1.0) if nm == "k"
                                  else (bq_sb, qTt, scale))
                if nm == "q":
                    dsts[mt][0] = qT_pool.tile([P, NQ], BF16, tag="qT",
                                               name=f"qT{mt}_0")
                nc.vector.tensor_scalar(
                    out=dsts[mt][0], in0=pss6[i],
                    scalar1=b_sb[:, mt:mt + 1], scalar2=sc,
                    op0=ALU.add, op1=ALU.mult,
                )
            kq_part(wq_t, bq_sb, qTt, scale, "q", 0, 2)
            kq_part(wq_t, bq_sb, qTt, scale, "q", 0, 3)

            def prologue_fill(kb):
                v_proj_tile(kb)

            def wave3_fill(mt):
                # During the exp-bound last wave, interleave all deferred
                # o_proj work: pieces of chunk mt during pair (mt,3) for
                # mt<3, the kt0..2 partials of chunk 3 during pair (3,3).
                def fill(kb):
                    if kb % 2 == 1:
                        i = kb // 2
                        if mt < 3:
                            o_proj_piece(mt, i // 2, i % 2)
                        else:
                            park_piece(i // 2, i % 2)
                return fill

            for qc in range(N_QC):
                if 1 <= qc < 3:
                    kq_dmas("k", xkT, qc + 1, nc.gpsimd)
                    kq_dmas("q", xqT, qc + 1, nc.gpsimd)
                    if qc == 1:
                        nc.gpsimd.dma_start(out=xvB[2],
                                            in_=xvT[:, 3, :, :])
                for mt in range(N_MT):
                    if qc == 0 and mt == 0:
                        attention_pair(0, 0, interleave=prologue_fill,
                                       warm=True)
                    elif qc == 3:
                        attention_pair(mt, 3, interleave=wave3_fill(mt))
                    else:
                        attention_pair(mt, qc, warm=(qc == 0))
                    if qc < 3:
                        nz = qc + 1
                        kq_part(wk_t, bk_sb, kTt, 1.0, "k", nz, mt)
                        v_proj_tile(4 * nz + mt)
                        kq_part(wq_t, bq_sb, qTt, scale, "q", nz, mt)
                flush_av()
            # ---- tail: the kt=3 contribution of the last wave's o_proj
            # joins the parked kt0..2 partial via an identity-matmul inject
            # (PSUM accumulate), so the only post-PE work per piece is one
            # copy -- alternated DVE / ACT, both idle here.
            for j in range(4):
                for dc in range(2):
                    lt = 12 + j
                    pool = ps_av if (2 * j + dc) % 2 else ps_mm
                    ps2 = pool.tile([P, NQ], F32,
                                    tag="av" if (2 * j + dc) % 2 else "mm",
                                    name=f"pof{j}_{dc}")
                    nc.tensor.matmul(
                        ps2,
                        lhsT=aoTq[3][3][:, j * P:(j + 1) * P],
                        rhs=wo_sb[:, 3, dc * NQ:(dc + 1) * NQ],
                        start=True,
                        stop=False,
                    )
                    nc.tensor.matmul(
                        ps2,
                        lhsT=ident_sb,
                        rhs=xk0[:, 2 * j + dc, :],
                        start=False,
                        stop=True,
                    )
                    ot = osb_pool.tile([P, NQ], BF16, tag="ot",
                                       name=f"ot{lt}_{dc}")
                    if dc == 0:
                        nc.vector.tensor_copy(out=ot, in_=ps2)
                    else:
                        nc.scalar.copy(out=ot, in_=ps2)
                    nc.sync.dma_start(
                        out=out[lt * P:(lt + 1) * P, dc * NQ:(dc + 1) * NQ],
                        in_=ot,
                    )
    nc.finalize()
    return nc


def make_in_maps(Q, K, V, Wq, bq, Wk, bk, Wv, bv, Wo, bo, attn_mask=None):
    """Build the 8 per-core input maps from full (unsharded) inputs."""
    Q = np.asarray(Q, np.float32)
    K = np.asarray(K, np.float32)
    V = np.asarray(V, np.float32)
    Wq = np.asarray(Wq, np.float32)
    Wk = np.asarray(Wk, np.float32)
    Wv = np.asarray(Wv, np.float32)
    Wo = np.asarray(Wo, np.float32)
    bq = np.asarray(bq, np.float32)
    bk = np.asarray(bk, np.float32)
    bv = np.asarray(bv, np.float32)

    i_idx = np.arange(P)[:, None]
    j_idx = np.arange(P)[None, :]
    tri = (i_idx <= j_idx).astype(NPBF16)
    maskc = np.concatenate([tri, tri], axis=1)  # duplicated for the 2 heads
    ident = np.eye(P, dtype=NPBF16)

    def x4(A):
        # [D, L] -> [P, N_QC, N_KT, NQ], 8KB-contiguous per (p, chunk)
        return np.ascontiguousarray(
            A.reshape(N_KT, P, N_QC, NQ).transpose(1, 2, 0, 3)
        ).astype(NPBF16)

    def w3(W):
        # [D, GD] -> [P, N_KT, GD]
        return np.ascontiguousarray(
            W.reshape(N_KT, P, -1).transpose(1, 0, 2)
        ).astype(NPBF16)

    def wo3(W):
        # [GD, D] -> [P, N_MT, D]
        return np.ascontiguousarray(
            W.reshape(N_MT, P, -1).transpose(1, 0, 2)
        ).astype(NPBF16)

    xT = {}
    for b in range(B):
        xT[b] = tuple(x4(X[b].T) for X in (Q, K, V))
    grp = {}
    for g in range(TP):
        sl = slice(g * GD, (g + 1) * GD)
        grp[g] = dict(
            wqT=w3(Wq[sl, :].T),
            wkT=w3(Wk[sl, :].T),
            wvT=w3(Wv[sl, :].T),
            woT=wo3(Wo[:, sl].T),
            bq=np.ascontiguousarray(bq[sl].reshape(N_MT, P).T).astype(np.float32),
            bk=np.ascontiguousarray(bk[sl].reshape(N_MT, P).T).astype(np.float32),
            bv=np.ascontiguousarray(bv[sl].reshape(1, GD)).astype(np.float32),
        )
    in_maps = []
    for c in range(2 * B):
        b, g = c // 2, c % 2
        m = dict(grp[g])
        m["xqT"], m["xkT"], m["xvT"] = xT[b]
        m["maskc"] = maskc
        m["ident"] = ident
        in_maps.append(m)
    return in_maps


def assemble_output(results, bo):
    bo = np.asarray(bo, np.float32)
    out = np.empty((B, L, D_MODEL), np.float32)
    for b in range(B):
        out[b] = (results[2 * b]["out"].astype(np.float32)
                  + results[2 * b + 1]["out"].astype(np.float32) + bo)
    return out


_NC_CACHE = None


def kernel(**inputs) -> np.ndarray:
    global _NC_CACHE
    from concourse.bass_utils import run_bass_kernel_spmd

    if _NC_CACHE is None:
        _NC_CACHE = build_nc()
    in_maps = make_in_maps(**inputs)
    res = run_bass_kernel_spmd(_NC_CACHE, in_maps, core_ids=list(range(2 * B)))
    return assemble_output(res.results, inputs["bo"])


# revision 51
# speedup vs baseline: 1.0226x; 1.0106x over previous
"""Multi-head causal attention (QKV proj + attention + O proj) on 8 TRN2 cores.

Sharding: data-parallel over batch (4) x tensor-parallel over heads (2 groups
of 8 heads).  Core c handles batch c//2, head-group c%2.  Each core computes
its group's partial o_proj output; the host sums the two partials per batch.

Layout strategy (all activations arrive pre-transposed from the host, so the
kernel never transposes on-device):
  - qT, kT per head-pair M-tile: (128 head-dims, L) from  W.T-slice @ X.T
  - v natural (tokens, head-dims) with a fused ones-column for the softmax
    denominator: av_psum = v_aug.T @ P.T gives (65, Nq) where row 64 is the
    per-query sum of probabilities.
  - scores are computed transposed (keys on partitions, queries free); the two
    heads of an M-tile run as concurrent PE row-tiles (K=64 at row 0 / 64).
  - AV emission is software-pipelined one key-block behind scores/exp so the
    strict-FIFO PE queue never stalls waiting on the ACT-engine exp.
  - causal diagonal blocks: exp first, then one fused tensor_tensor multiply
    (both heads at once) against a host-duplicated triangular mask.
  - softmax denominators: fast-approx reciprocal on the 1-partition row (bf16),
    then a K=1 PE matmul (ones outer product) broadcasts it to partitions
    64..127 of a PSUM bank -- no DRAM bounce.
Schedule: the prologue DMAs are sliced per contraction tile and spread over
three engine queues in need-order (vector: weights, sync: x chunks, scalar:
tiny constants) so projection chains start within ~3us; later x chunks are
gated behind the critical phase.  All o_proj work is deferred to the last
query-chunk wave, which is otherwise ACT(exp)-bound, so the PE has filler
there.  Outputs are stored bf16 (host accumulates in fp32).
Compute dtype bf16 (fp32 PSUM accumulation); bf16 partial outputs.
"""

import numpy as np
import ml_dtypes

import concourse.bass as bass
import concourse.tile as tile
from concourse import bacc, mybir
from concourse.tile import add_dep_helper

D_MODEL = 1024
N_HEADS = 16
D_K = 64
B, L = 4, 2048
TP = 2                  # head groups
GD = D_MODEL // TP      # 512 head-dims per group
P = 128
NQ = 512                # query chunk (one fp32 PSUM bank)
N_MT = GD // P          # 4 M-tiles (head pairs) per group
N_KT = D_MODEL // P     # 8 contraction tiles over model dim
N_TT = L // P           # 16 token tiles
N_QC = L // NQ          # 4 query chunks
BF16 = mybir.dt.bfloat16
F32 = mybir.dt.float32
NPBF16 = ml_dtypes.bfloat16
AF = mybir.ActivationFunctionType
ALU = mybir.AluOpType


def build_nc() -> bass.Bass:
    nc = bacc.Bacc("TRN2", target_bir_lowering=False)

    # All bulk inputs arrive in partition-major layouts with >=8KB of
    # contiguous data per partition line -- 1KB lines measure ~50-90GB/s
    # per DMA queue, 8KB lines ~250GB/s.
    xqT = nc.dram_tensor("xqT", [P, N_QC, N_KT, NQ], BF16, kind="ExternalInput")
    xkT = nc.dram_tensor("xkT", [P, N_QC, N_KT, NQ], BF16, kind="ExternalInput")
    xvT = nc.dram_tensor("xvT", [P, N_QC, N_KT, NQ], BF16, kind="ExternalInput")
    wqT = nc.dram_tensor("wqT", [P, N_KT, GD], BF16, kind="ExternalInput")
    wkT = nc.dram_tensor("wkT", [P, N_KT, GD], BF16, kind="ExternalInput")
    wvT = nc.dram_tensor("wvT", [P, N_KT, GD], BF16, kind="ExternalInput")
    woT = nc.dram_tensor("woT", [P, N_MT, D_MODEL], BF16, kind="ExternalInput")
    bq = nc.dram_tensor("bq", [P, N_MT], F32, kind="ExternalInput")
    bk = nc.dram_tensor("bk", [P, N_MT], F32, kind="ExternalInput")
    bv = nc.dram_tensor("bv", [1, GD], F32, kind="ExternalInput")
    maskc = nc.dram_tensor("maskc", [P, 2 * P], BF16, kind="ExternalInput")
    ident = nc.dram_tensor("ident", [P, P], BF16, kind="ExternalInput")
    out = nc.dram_tensor("out", [L, D_MODEL], BF16, kind="ExternalOutput")

    with tile.TileContext(nc) as tc:
        with (
            tc.tile_pool(name="const", bufs=1) as const,
            tc.tile_pool(name="xch", bufs=3) as xch_pool,
            tc.tile_pool(name="qT", bufs=9) as qT_pool,
            tc.tile_pool(name="pt", bufs=4) as pt_pool,
            tc.tile_pool(name="small", bufs=2) as small_pool,
            tc.tile_pool(name="osb", bufs=2) as osb_pool,
            tc.tile_pool(name="acc", bufs=4) as acc_pool,
            tc.tile_pool(name="ps_s", bufs=2, space="PSUM") as ps_s,
            tc.tile_pool(name="ps_av", bufs=2, space="PSUM") as ps_av,
            tc.tile_pool(name="ps_mm", bufs=2, space="PSUM") as ps_mm,
        ):
            scale = float(1.0 / np.sqrt(np.float32(D_K)))

            # memsets first so the warmup matmuls and the reciprocal
            # broadcast constant are ready immediately (DVE queue head)
            w_warm = const.tile([P, NQ], BF16, tag="warm")
            nc.vector.memset(w_warm, 0.125)
            ones64 = const.tile([1, D_K], BF16, tag="ones64")
            nc.vector.memset(ones64, 1.0)

            # PE clock warmup: a few dependency-free matmuls keep the HAM
            # clock gate from seeing a dead PE while the first DMAs land.
            for i in range(6):
                ps_w = ps_mm.tile([P, NQ], F32, tag="mm", name=f"warm{i}")
                nc.tensor.matmul(ps_w, lhsT=w_warm[:, 0:P], rhs=w_warm,
                                 start=True, stop=True)

            # ---- tiny constants first (scalar queue; ACT engine is idle now)
            bk_sb = const.tile([P, N_MT], F32, tag="bk")
            bq_sb = const.tile([P, N_MT], F32, tag="bq")
            bv_sb = const.tile([P, GD], F32, tag="bv")
            mask_sb = const.tile([P, 2, P], BF16, tag="mask")
            nc.scalar.dma_start(out=bk_sb, in_=bk[:, :])
            nc.scalar.dma_start(out=bq_sb, in_=bq[:, :])

            # ---- prologue bulk: one whole-tensor DMA each (8KB partition
            # lines: 1KB lines measure ~50-90GB/s per queue, 8KB ~250GB/s),
            # spread across the three queues in need-order.  The DVE queue
            # stays free for the kT/qT epilogues and v bias adds.
            H = N_KT // 2
            wk_h = [const.tile([P, H, GD], BF16, tag=f"wk{h}", name=f"wk{h}")
                    for h in range(2)]
            xk0_h = [const.tile([P, H, NQ], BF16, tag=f"xk0{h}", name=f"xk0{h}")
                     for h in range(2)]
            wq_sb = const.tile([P, N_KT, GD], BF16, tag="wq")
            wv_sb = const.tile([P, N_KT, GD], BF16, tag="wv")
            wo_sb = const.tile([P, N_MT, D_MODEL], BF16, tag="wo")
            xq0 = const.tile([P, N_KT, NQ], BF16, tag="xq0")
            xv0 = const.tile([P, N_KT, NQ], BF16, tag="xv0")

            # k-path first on sync (fastest queue), in half-tensor
            # granularity so the kt-major chains start at ~9us
            nc.sync.dma_start(out=wk_h[0], in_=wkT[:, 0:H, :])
            nc.sync.dma_start(out=xk0_h[0], in_=xkT[:, 0, 0:H, :])
            nc.sync.dma_start(out=wk_h[1], in_=wkT[:, H:N_KT, :])
            nc.sync.dma_start(out=xk0_h[1], in_=xkT[:, 0, H:N_KT, :])
            nc.sync.dma_start(out=xq0, in_=xqT[:, 0, :, :])
            nc.gpsimd.dma_start(out=wq_sb, in_=wqT[:, :, :])
            nc.gpsimd.dma_start(out=wv_sb, in_=wvT[:, :, :])
            nc.scalar.dma_start(out=mask_sb,
                                in_=maskc.rearrange("p (h n) -> p h n", n=P))
            h_gate = nc.scalar.dma_start(out=xv0, in_=xvT[:, 0, :, :])
            nc.scalar.dma_start(out=bv_sb, in_=bv[:, :].to_broadcast([P, GD]))
            ident_sb = const.tile([P, P], BF16, tag="ident")
            nc.scalar.dma_start(out=ident_sb, in_=ident[:, :])
            wk_t = [wk_h[kt // H][:, kt % H, :] for kt in range(N_KT)]
            wq_t = [wq_sb[:, kt, :] for kt in range(N_KT)]
            wv_t = [wv_sb[:, kt, :] for kt in range(N_KT)]

            xvB = [const.tile([P, N_KT, NQ], BF16, tag=f"xvb{i}", name=f"xvb{i}")
                   for i in range(3)]

            xch_cache = {
                ("k", 0): [xk0_h[kt // H][:, kt % H, :] for kt in range(N_KT)],
                ("q", 0): [xq0[:, kt, :] for kt in range(N_KT)],
            }
            # dead xk0 halves double as bf16 scratch for the o_proj parks
            park_sb = [xk0_h[i // H][:, i % H, :] for i in range(N_KT)]

            def kq_dmas(nm, x_dram, ncz, eng):
                xc = xch_pool.tile([P, N_KT, NQ], BF16, tag="xch",
                                   name=f"x{nm}{ncz}")
                h = eng.dma_start(out=xc, in_=x_dram[:, ncz, :, :])
                xch_cache[(nm, ncz)] = [xc[:, kt, :] for kt in range(N_KT)]
                return h

            # chunk-1 loads: k on the scalar queue (idle after the critical
            # weights), q + wo behind the critical x on sync, xvB0 on gpsimd.
            kq_dmas("k", xkT, 1, nc.scalar)
            kq_dmas("q", xqT, 1, nc.sync)
            nc.sync.dma_start(out=wo_sb, in_=woT[:, :, :])
            nc.gpsimd.dma_start(out=xvB[0], in_=xvT[:, 1, :, :])

            # later x loads stay on the gpsimd queue, gated behind the
            # critical phase so they don't steal HBM bandwidth from it.
            h1 = nc.gpsimd.dma_start(out=xvB[1], in_=xvT[:, 2, :, :])
            add_dep_helper(h1.ins, h_gate.ins, sync=True,
                           reason="late xv chunks wait for critical phase")

            def v_proj_tile(tt):
                ps = ps_mm.tile([P, GD], F32, tag="mm", name=f"psv{tt}")
                for kt in range(N_KT):
                    if tt < 4:
                        xs = xv0[:, kt, (tt % 4) * P:(tt % 4 + 1) * P]
                    else:
                        xs = xvB[tt // 4 - 1][:, kt, (tt % 4) * P:(tt % 4 + 1) * P]
                    nc.tensor.matmul(
                        ps,
                        lhsT=xs,
                        rhs=wv_t[kt],
                        start=(kt == 0),
                        stop=(kt == N_KT - 1),
                    )
                nc.vector.tensor_tensor(
                    out=vA[tt][:, :, VW - D_K:VW],
                    in0=ps.rearrange("p (h d) -> p h d", d=D_K),
                    in1=bv_sb.rearrange("p (h d) -> p h d", d=D_K),
                    op=ALU.add,
                )
                nc.vector.memset(vA[tt][:, :, 1:VW - D_K], 0.0)
                nc.vector.memset(vA[tt][:, :, 0:1], 1.0)

            def kq_part(w_t, b_sb, dsts, sc, nm, ncz, mt):
                xchs = xch_cache[(nm, ncz)]
                if nm == "q":
                    dsts[mt][ncz] = qT_pool.tile([P, NQ], BF16, tag="qT",
                                                 name=f"qT{mt}_{ncz}")
                ps = ps_mm.tile([P, NQ], F32, tag="mm", name=f"ps{nm}{ncz}{mt}")
                for kt in range(N_KT):
                    nc.tensor.matmul(
                        ps,
                        lhsT=w_t[kt][:, mt * P:(mt + 1) * P],
                        rhs=xchs[kt],
                        start=(kt == 0),
                        stop=(kt == N_KT - 1),
                    )
                nc.vector.tensor_scalar(
                    out=dsts[mt][ncz],
                    in0=ps,
                    scalar1=b_sb[:, mt:mt + 1],
                    scalar2=sc,
                    op0=ALU.add,
                    op1=ALU.mult,
                )

            # per-(mt, chunk) tiles so consumers unblock as soon as possible
            qTt = [[None for _ in range(N_QC)] for _ in range(N_MT)]
            kTt = [[const.tile([P, NQ], BF16, tag=f"kT{mt}_{ncz}", name=f"kT{mt}_{ncz}")
                    for ncz in range(N_QC)] for mt in range(N_MT)]
            # 128 cols per head: [ones, 63 zeros, 64 V-dims] so the AV output
            # puts the denominator at partition 0 and V rows at partition 64
            VW = 128
            vA = [const.tile([P, 2 * N_MT, VW], BF16, tag=f"v{tt}", name=f"v{tt}")
                  for tt in range(N_TT)]
            aoTq = [[const.tile([P, NQ], BF16, tag=f"ao{mt}_{qc}", name=f"ao{mt}_{qc}")
                     for qc in range(N_QC)] for mt in range(N_MT)]

            # ---- attention, software-pipelined: the AV pair for a key-block
            # is emitted two blocks after its scores/exp, so the PE (strict
            # FIFO queue) has scores work while ACT runs exp and never stalls
            # on a single exp's latency.
            pipe = {"q": []}

            def normalize(mt, qc, acc):
                for h2 in range(2):
                    rec1 = small_pool.tile([1, NQ], F32, tag="rec1",
                                           name=f"rec1{mt}_{qc}_{h2}")
                    nc.vector.reciprocal_approx_fast(
                        out=rec1, in_=acc[h2][0:1, :])
                    recb = small_pool.tile([1, NQ], BF16, tag="recb",
                                           name=f"recb{mt}_{qc}_{h2}")
                    nc.vector.tensor_copy(out=recb, in_=rec1)
                    # broadcast to partitions 64..127 via a K=1 outer product
                    # on the PE (partition-broadcast DMA is broken on HW, and
                    # a DRAM bounce costs ~3us of latency + 2 DMAs)
                    bc = ps_mm.tile([P, NQ], F32, tag="mm",
                                    name=f"bc{mt}_{qc}_{h2}")
                    nc.tensor.matmul(bc[VW - D_K:VW, :], lhsT=ones64[0:1, :],
                                     rhs=recb, start=True, stop=True)
                    nc.vector.tensor_tensor(
                        out=aoTq[mt][qc][h2 * D_K:(h2 + 1) * D_K, :],
                        in0=acc[h2][VW - D_K:VW, :],
                        in1=bc[VW - D_K:VW, :],
                        op=ALU.mult,
                    )

            def emit_av(st):
                mt, qc, kb, p3, av, acc = st
                nkb = 4 * qc + 4
                t = P * (kb - 4 * qc)
                grp_start = (kb % 8 == 0)
                grp_stop = (kb % 8 == 7) or (kb == nkb - 1)
                for h2 in range(2):
                    nc.tensor.matmul(
                        av[h2][:, max(t, 0):NQ],
                        lhsT=vA[kb][:, 2 * mt + h2, :],
                        rhs=p3[:, h2, max(t, 0):NQ],
                        start=grp_start,
                        stop=grp_stop,
                    )
                if grp_stop:  # evict group into SBUF accumulator
                    for h2 in range(2):
                        if kb < 8:
                            nc.vector.tensor_copy(out=acc[h2], in_=av[h2])
                        else:
                            nc.vector.tensor_tensor(
                                out=acc[h2], in0=acc[h2], in1=av[h2],
                                op=ALU.add,
                            )
                if kb == nkb - 1:
                    # pair complete: normalize here so the DVE queue sees the
                    # eviction before the ops that consume the accumulator.
                    normalize(mt, qc, acc)

            def attention_pair(mt, qc, interleave=None, warm=False):
                acc = [acc_pool.tile([VW, NQ], F32, tag="acc",
                                     name=f"acc{mt}_{qc}_{i}") for i in range(2)]
                av = [None, None]
                nkb = 4 * qc + 4
                for kb in range(nkb):
                    if kb % 8 == 0:
                        av = [ps_av.tile([VW, NQ], F32, tag="av",
                                         name=f"av{mt}_{qc}_{kb}_{i}")
                              for i in range(2)]
                    t = P * (kb - 4 * qc)  # <0 for full blocks
                    s_ps = ps_s.tile([P, 2 * NQ], F32, tag="s",
                                     name=f"s{mt}_{qc}_{kb}")
                    s3 = s_ps.rearrange("p (h n) -> p h n", n=NQ)
                    for h2 in range(2):
                        nc.tensor.matmul(
                            s3[:, h2, max(t, 0):NQ],
                            lhsT=kTt[mt][kb // 4][h2 * D_K:(h2 + 1) * D_K,
                                                 (kb % 4) * P:(kb % 4 + 1) * P],
                            rhs=qTt[mt][qc][h2 * D_K:(h2 + 1) * D_K,
                                            max(t, 0):NQ],
                            start=True,
                            stop=True,
                        )
                    pt = pt_pool.tile([P, 2 * NQ], BF16, tag="pt",
                                      name=f"pt{mt}_{qc}_{kb}")
                    p3 = pt.rearrange("p (h n) -> p h n", n=NQ)
                    if t <= 0:
                        nc.scalar.activation(out=pt, in_=s_ps, func=AF.Exp)
                    else:
                        nc.scalar.activation(out=p3[:, :, t:NQ],
                                             in_=s3[:, :, t:NQ], func=AF.Exp)
                    if t >= 0:  # diagonal sub-block: fused triangular mask
                        nc.vector.tensor_tensor(
                            out=p3[:, :, t:t + P],
                            in0=p3[:, :, t:t + P],
                            in1=mask_sb,
                            op=ALU.mult,
                        )
                    pipe["q"].append((mt, qc, kb, p3, av, acc))
                    if len(pipe["q"]) > 2:
                        emit_av(pipe["q"].pop(0))
                    if interleave is not None:
                        interleave(kb)

            def flush_av():
                while pipe["q"]:
                    emit_av(pipe["q"].pop(0))

            def o_proj_piece(qc, j, dc):
                lt = 4 * qc + j
                ps = ps_mm.tile([P, NQ], F32, tag="mm", name=f"po{lt}_{dc}")
                for kt in range(N_MT):
                    nc.tensor.matmul(
                        ps,
                        lhsT=aoTq[kt][qc][:, j * P:(j + 1) * P],
                        rhs=wo_sb[:, kt, dc * NQ:(dc + 1) * NQ],
                        start=(kt == 0),
                        stop=(kt == N_MT - 1),
                    )
                ot = osb_pool.tile([P, NQ], BF16, tag="ot", name=f"ot{lt}_{dc}")
                nc.vector.tensor_copy(out=ot, in_=ps)
                nc.sync.dma_start(
                    out=out[lt * P:(lt + 1) * P, dc * NQ:(dc + 1) * NQ],
                    in_=ot,
                )

            def park_piece(j, dc):
                # o_proj for the last wave, head-pairs 0..2 only; the kt=3
                # contribution joins after normalize(3,3) in the tail.
                pso = ps_mm.tile([P, NQ], F32, tag="mm", name=f"pop{j}_{dc}")
                for kt in range(3):
                    nc.tensor.matmul(
                        pso,
                        lhsT=aoTq[kt][3][:, j * P:(j + 1) * P],
                        rhs=wo_sb[:, kt, dc * NQ:(dc + 1) * NQ],
                        start=(kt == 0),
                        stop=(kt == 2),
                    )
                nc.vector.tensor_copy(out=park_sb[2 * j + dc], in_=pso)

            # ---- emission schedule.  Prologue: six chunk-0 chains run
            # kt-MAJOR (their matmuls interleaved slice-by-slice), borrowing
            # the idle ps_s/ps_av banks, so every arriving x-slice unblocks
            # six matmuls instead of one -- the strict-FIFO PE queue never
            # waits on one chain's slowest slice.  v_proj tiles (whose
            # wv/xv0 data lands last) interleave into pair (0,0).
            six = [
                ("k", 0, ps_s, "s"), ("q", 0, ps_s, "s"),
                ("k", 1, ps_av, "av"), ("q", 1, ps_av, "av"),
                ("k", 2, ps_mm, "mm"), ("k", 3, ps_mm, "mm"),
            ]
            pss6 = [pool.tile([P, NQ], F32, tag=tag, name=f"pc{nm}{mt}")
                    for nm, mt, pool, tag in six]
            for kt in range(N_KT):
                for i, (nm, mt, pool, tag) in enumerate(six):
                    w_t = wk_t if nm == "k" else wq_t
                    nc.tensor.matmul(
                        pss6[i],
                        lhsT=w_t[kt][:, mt * P:(mt + 1) * P],
                        rhs=xch_cache[(nm, 0)][kt],
                        start=(kt == 0),
                        stop=(kt == N_KT - 1),
                    )
            for i, (nm, mt, pool, tag) in enumerate(six):
                b_sb, dsts, sc = ((bk_sb, kTt, 1.0) if nm == "k"
                                  else (bq_sb, qTt, scale))
                if nm == "q":
                    dsts[mt][0] = qT_pool.tile([P, NQ], BF16, tag="qT",
                                               name=f"qT{mt}_0")
                nc.vector.tensor_scalar(
                    out=dsts[mt][0], in0=pss6[i],
                    scalar1=b_sb[:, mt:mt + 1], scalar2=sc,
                    op0=ALU.add, op1=ALU.mult,
                )
            kq_part(wq_t, bq_sb, qTt, scale, "q", 0, 2)
            kq_part(wq_t, bq_sb, qTt, scale, "q", 0, 3)

            def prologue_fill(kb):
                v_proj_tile(kb)

            def wave3_fill(mt):
                # During the exp-bound last wave, interleave all deferred
                # o_proj work: pieces of chunk mt during pair (mt,3) for
                # mt<3, the kt0..2 partials of chunk 3 during pair (3,3).
                def fill(kb):
                    if kb % 2 == 1:
                        i = kb // 2
                        if mt < 3:
                            o_proj_piece(mt, i // 2, i % 2)
                        else:
                            park_piece(i // 2, i % 2)
                return fill

            for qc in range(N_QC):
                if 1 <= qc < 3:
                    kq_dmas("k", xkT, qc + 1, nc.gpsimd)
                    kq_dmas("q", xqT, qc + 1, nc.gpsimd)
                    if qc == 1:
                        nc.gpsimd.dma_start(out=xvB[2],
                                            in_=xvT[:, 3, :, :])
                for mt in range(N_MT):
                    if qc == 0 and mt == 0:
                        attention_pair(0, 0, interleave=prologue_fill,
                                       warm=True)
                    elif qc == 3:
                        attention_pair(mt, 3, interleave=wave3_fill(mt))
                    else:
                        attention_pair(mt, qc, warm=(qc == 0))
                    if qc < 3:
                        nz = qc + 1
                        kq_part(wk_t, bk_sb, kTt, 1.0, "k", nz, mt)
                        v_proj_tile(4 * nz + mt)
                        kq_part(wq_t, bq_sb, qTt, scale, "q", nz, mt)
                flush_av()
            # ---- tail: the kt=3 contribution of the last wave's o_proj
            # joins the parked kt0..2 partial via an identity-matmul inject
            # (PSUM accumulate), so the only post-PE work per piece is one
            # copy -- alternated DVE / ACT, both idle here.
            for j in range(4):
                for dc in range(2):
                    lt = 12 + j
                    pool = ps_av if (2 * j + dc) % 2 else ps_mm
                    ps2 = pool.tile([P, NQ], F32,
                                    tag="av" if (2 * j + dc) % 2 else "mm",
                                    name=f"pof{j}_{dc}")
                    nc.tensor.matmul(
                        ps2,
                        lhsT=aoTq[3][3][:, j * P:(j + 1) * P],
                        rhs=wo_sb[:, 3, dc * NQ:(dc + 1) * NQ],
                        start=True,
                        stop=False,
                    )
                    nc.tensor.matmul(
                        ps2,
                        lhsT=ident_sb,
                        rhs=park_sb[2 * j + dc],
                        start=False,
                        stop=True,
                    )
                    ot = osb_pool.tile([P, NQ], BF16, tag="ot",
                                       name=f"ot{lt}_{dc}")
                    if dc == 0:
                        nc.vector.tensor_copy(out=ot, in_=ps2)
                    else:
                        nc.scalar.copy(out=ot, in_=ps2)
                    nc.sync.dma_start(
                        out=out[lt * P:(lt + 1) * P, dc * NQ:(dc + 1) * NQ],
                        in_=ot,
                    )
    nc.finalize()
    return nc


def make_in_maps(Q, K, V, Wq, bq, Wk, bk, Wv, bv, Wo, bo, attn_mask=None):
    """Build the 8 per-core input maps from full (unsharded) inputs."""
    Q = np.asarray(Q, np.float32)
    K = np.asarray(K, np.float32)
    V = np.asarray(V, np.float32)
    Wq = np.asarray(Wq, np.float32)
    Wk = np.asarray(Wk, np.float32)
    Wv = np.asarray(Wv, np.float32)
    Wo = np.asarray(Wo, np.float32)
    bq = np.asarray(bq, np.float32)
    bk = np.asarray(bk, np.float32)
    bv = np.asarray(bv, np.float32)

    i_idx = np.arange(P)[:, None]
    j_idx = np.arange(P)[None, :]
    tri = (i_idx <= j_idx).astype(NPBF16)
    maskc = np.concatenate([tri, tri], axis=1)  # duplicated for the 2 heads
    ident = np.eye(P, dtype=NPBF16)

    def x4(A):
        # [D, L] -> [P, N_QC, N_KT, NQ], 8KB-contiguous per (p, chunk)
        return np.ascontiguousarray(
            A.reshape(N_KT, P, N_QC, NQ).transpose(1, 2, 0, 3)
        ).astype(NPBF16)

    def w3(W):
        # [D, GD] -> [P, N_KT, GD]
        return np.ascontiguousarray(
            W.reshape(N_KT, P, -1).transpose(1, 0, 2)
        ).astype(NPBF16)

    def wo3(W):
        # [GD, D] -> [P, N_MT, D]
        return np.ascontiguousarray(
            W.reshape(N_MT, P, -1).transpose(1, 0, 2)
        ).astype(NPBF16)

    xT = {}
    for b in range(B):
        xT[b] = tuple(x4(X[b].T) for X in (Q, K, V))
    grp = {}
    for g in range(TP):
        sl = slice(g * GD, (g + 1) * GD)
        grp[g] = dict(
            wqT=w3(Wq[sl, :].T),
            wkT=w3(Wk[sl, :].T),
            wvT=w3(Wv[sl, :].T),
            woT=wo3(Wo[:, sl].T),
            bq=np.ascontiguousarray(bq[sl].reshape(N_MT, P).T).astype(np.float32),
            bk=np.ascontiguousarray(bk[sl].reshape(N_MT, P).T).astype(np.float32),
            bv=np.ascontiguousarray(bv[sl].reshape(1, GD)).astype(np.float32),
        )
    in_maps = []
    for c in range(2 * B):
        b, g = c // 2, c % 2
        m = dict(grp[g])
        m["xqT"], m["xkT"], m["xvT"] = xT[b]
        m["maskc"] = maskc
        m["ident"] = ident
        in_maps.append(m)
    return in_maps


def assemble_output(results, bo):
    bo = np.asarray(bo, np.float32)
    out = np.empty((B, L, D_MODEL), np.float32)
    for b in range(B):
        out[b] = (results[2 * b]["out"].astype(np.float32)
                  + results[2 * b + 1]["out"].astype(np.float32) + bo)
    return out


_NC_CACHE = None


def kernel(**inputs) -> np.ndarray:
    global _NC_CACHE
    from concourse.bass_utils import run_bass_kernel_spmd

    if _NC_CACHE is None:
        _NC_CACHE = build_nc()
    in_maps = make_in_maps(**inputs)
    res = run_bass_kernel_spmd(_NC_CACHE, in_maps, core_ids=list(range(2 * B)))
    return assemble_output(res.results, inputs["bo"])


# revision 53
# speedup vs baseline: 1.0259x; 1.0032x over previous
"""Multi-head causal attention (QKV proj + attention + O proj) on 8 TRN2 cores.

Sharding: data-parallel over batch (4) x tensor-parallel over heads (2 groups
of 8 heads).  Core c handles batch c//2, head-group c%2.  Each core computes
its group's partial o_proj output; the host sums the two partials per batch.

Layout strategy (all activations arrive pre-transposed from the host, so the
kernel never transposes on-device):
  - qT, kT per head-pair M-tile: (128 head-dims, L) from  W.T-slice @ X.T
  - v natural (tokens, head-dims) with a fused ones-column for the softmax
    denominator: av_psum = v_aug.T @ P.T gives (65, Nq) where row 64 is the
    per-query sum of probabilities.
  - scores are computed transposed (keys on partitions, queries free); the two
    heads of an M-tile run as concurrent PE row-tiles (K=64 at row 0 / 64).
  - AV emission is software-pipelined one key-block behind scores/exp so the
    strict-FIFO PE queue never stalls waiting on the ACT-engine exp.
  - causal diagonal blocks: exp first, then one fused tensor_tensor multiply
    (both heads at once) against a host-duplicated triangular mask.
  - softmax denominators: fast-approx reciprocal on the 1-partition row (bf16),
    then a K=1 PE matmul (ones outer product) broadcasts it to partitions
    64..127 of a PSUM bank -- no DRAM bounce.
Schedule: the prologue DMAs are sliced per contraction tile and spread over
three engine queues in need-order (vector: weights, sync: x chunks, scalar:
tiny constants) so projection chains start within ~3us; later x chunks are
gated behind the critical phase.  All o_proj work is deferred to the last
query-chunk wave, which is otherwise ACT(exp)-bound, so the PE has filler
there.  Outputs are stored bf16 (host accumulates in fp32).
Compute dtype bf16 (fp32 PSUM accumulation); bf16 partial outputs.
"""

import numpy as np
import ml_dtypes

import concourse.bass as bass
import concourse.tile as tile
from concourse import bacc, mybir
from concourse.tile import add_dep_helper

D_MODEL = 1024
N_HEADS = 16
D_K = 64
B, L = 4, 2048
TP = 2                  # head groups
GD = D_MODEL // TP      # 512 head-dims per group
P = 128
NQ = 512                # query chunk (one fp32 PSUM bank)
N_MT = GD // P          # 4 M-tiles (head pairs) per group
N_KT = D_MODEL // P     # 8 contraction tiles over model dim
N_TT = L // P           # 16 token tiles
N_QC = L // NQ          # 4 query chunks
BF16 = mybir.dt.bfloat16
F32 = mybir.dt.float32
NPBF16 = ml_dtypes.bfloat16
AF = mybir.ActivationFunctionType
ALU = mybir.AluOpType


def build_nc() -> bass.Bass:
    nc = bacc.Bacc("TRN2", target_bir_lowering=False)

    # All bulk inputs arrive in partition-major layouts with >=8KB of
    # contiguous data per partition line -- 1KB lines measure ~50-90GB/s
    # per DMA queue, 8KB lines ~250GB/s.
    xqT = nc.dram_tensor("xqT", [P, N_QC, N_KT, NQ], BF16, kind="ExternalInput")
    xkT = nc.dram_tensor("xkT", [P, N_QC, N_KT, NQ], BF16, kind="ExternalInput")
    xvT = nc.dram_tensor("xvT", [P, N_QC, N_KT, NQ], BF16, kind="ExternalInput")
    wqT = nc.dram_tensor("wqT", [P, N_KT, GD], BF16, kind="ExternalInput")
    wkT = nc.dram_tensor("wkT", [P, N_KT, GD], BF16, kind="ExternalInput")
    wvT = nc.dram_tensor("wvT", [P, N_KT, GD], BF16, kind="ExternalInput")
    woT = nc.dram_tensor("woT", [P, N_MT, D_MODEL], BF16, kind="ExternalInput")
    bq = nc.dram_tensor("bq", [P, N_MT], F32, kind="ExternalInput")
    bk = nc.dram_tensor("bk", [P, N_MT], F32, kind="ExternalInput")
    bv = nc.dram_tensor("bv", [1, GD], F32, kind="ExternalInput")
    maskc = nc.dram_tensor("maskc", [P, 2 * P], BF16, kind="ExternalInput")
    ident = nc.dram_tensor("ident", [P, P], BF16, kind="ExternalInput")
    out = nc.dram_tensor("out", [L, D_MODEL], BF16, kind="ExternalOutput")

    with tile.TileContext(nc) as tc:
        with (
            tc.tile_pool(name="const", bufs=1) as const,
            tc.tile_pool(name="xch", bufs=3) as xch_pool,
            tc.tile_pool(name="qT", bufs=9) as qT_pool,
            tc.tile_pool(name="pt", bufs=4) as pt_pool,
            tc.tile_pool(name="small", bufs=2) as small_pool,
            tc.tile_pool(name="osb", bufs=2) as osb_pool,
            tc.tile_pool(name="acc", bufs=4) as acc_pool,
            tc.tile_pool(name="ps_s", bufs=2, space="PSUM") as ps_s,
            tc.tile_pool(name="ps_av", bufs=2, space="PSUM") as ps_av,
            tc.tile_pool(name="ps_mm", bufs=2, space="PSUM") as ps_mm,
        ):
            scale = float(1.0 / np.sqrt(np.float32(D_K)))

            # memsets first so the warmup matmuls and the reciprocal
            # broadcast constant are ready immediately (DVE queue head)
            w_warm = const.tile([P, NQ], BF16, tag="warm")
            nc.vector.memset(w_warm, 0.125)
            ones64 = const.tile([1, D_K], BF16, tag="ones64")
            nc.vector.memset(ones64, 1.0)

            # PE clock warmup: a few dependency-free matmuls keep the HAM
            # clock gate from seeing a dead PE while the first DMAs land.
            for i in range(6):
                ps_w = ps_mm.tile([P, NQ], F32, tag="mm", name=f"warm{i}")
                nc.tensor.matmul(ps_w, lhsT=w_warm[:, 0:P], rhs=w_warm,
                                 start=True, stop=True)

            # ---- tiny constants first (scalar queue; ACT engine is idle now)
            bk_sb = const.tile([P, N_MT], F32, tag="bk")
            bq_sb = const.tile([P, N_MT], F32, tag="bq")
            bv_sb = const.tile([P, GD], F32, tag="bv")
            mask_sb = const.tile([P, 2, P], BF16, tag="mask")
            nc.scalar.dma_start(out=bk_sb, in_=bk[:, :])
            nc.scalar.dma_start(out=bq_sb, in_=bq[:, :])

            # ---- prologue bulk: one whole-tensor DMA each (8KB partition
            # lines: 1KB lines measure ~50-90GB/s per queue, 8KB ~250GB/s),
            # spread across the three queues in need-order.  The DVE queue
            # stays free for the kT/qT epilogues and v bias adds.
            H = N_KT // 2
            wk_h = [const.tile([P, H, GD], BF16, tag=f"wk{h}", name=f"wk{h}")
                    for h in range(2)]
            xk0_h = [const.tile([P, H, NQ], BF16, tag=f"xk0{h}", name=f"xk0{h}")
                     for h in range(2)]
            wq_sb = const.tile([P, N_KT, GD], BF16, tag="wq")
            wv_sb = const.tile([P, N_KT, GD], BF16, tag="wv")
            wo_sb = const.tile([P, N_MT, D_MODEL], BF16, tag="wo")
            xq0 = const.tile([P, N_KT, NQ], BF16, tag="xq0")
            xv0 = const.tile([P, N_KT, NQ], BF16, tag="xv0")

            # k-path first on sync (fastest queue), in half-tensor
            # granularity so the kt-major chains start at ~9us
            nc.sync.dma_start(out=wk_h[0], in_=wkT[:, 0:H, :])
            nc.sync.dma_start(out=xk0_h[0], in_=xkT[:, 0, 0:H, :])
            nc.sync.dma_start(out=wk_h[1], in_=wkT[:, H:N_KT, :])
            nc.sync.dma_start(out=xk0_h[1], in_=xkT[:, 0, H:N_KT, :])
            nc.sync.dma_start(out=xq0, in_=xqT[:, 0, :, :])
            nc.gpsimd.dma_start(out=wq_sb, in_=wqT[:, :, :])
            nc.gpsimd.dma_start(out=wv_sb, in_=wvT[:, :, :])
            nc.scalar.dma_start(out=mask_sb,
                                in_=maskc.rearrange("p (h n) -> p h n", n=P))
            h_gate = nc.scalar.dma_start(out=xv0, in_=xvT[:, 0, :, :])
            nc.scalar.dma_start(out=bv_sb, in_=bv[:, :].to_broadcast([P, GD]))
            ident_sb = const.tile([P, P], BF16, tag="ident")
            nc.scalar.dma_start(out=ident_sb, in_=ident[:, :])
            wk_t = [wk_h[kt // H][:, kt % H, :] for kt in range(N_KT)]
            wq_t = [wq_sb[:, kt, :] for kt in range(N_KT)]
            wv_t = [wv_sb[:, kt, :] for kt in range(N_KT)]

            xvB = [const.tile([P, N_KT, NQ], BF16, tag=f"xvb{i}", name=f"xvb{i}")
                   for i in range(3)]

            xch_cache = {
                ("k", 0): [xk0_h[kt // H][:, kt % H, :] for kt in range(N_KT)],
                ("q", 0): [xq0[:, kt, :] for kt in range(N_KT)],
            }
            # dead xk0 halves double as bf16 scratch for the o_proj parks
            park_sb = [xk0_h[i // H][:, i % H, :] for i in range(N_KT)]

            def kq_dmas(nm, x_dram, ncz, eng):
                xc = xch_pool.tile([P, N_KT, NQ], BF16, tag="xch",
                                   name=f"x{nm}{ncz}")
                h = eng.dma_start(out=xc, in_=x_dram[:, ncz, :, :])
                xch_cache[(nm, ncz)] = [xc[:, kt, :] for kt in range(N_KT)]
                return h

            # chunk-1 loads: k on the scalar queue (idle after the critical
            # weights), q + wo behind the critical x on sync, xvB0 on gpsimd.
            kq_dmas("k", xkT, 1, nc.scalar)
            kq_dmas("q", xqT, 1, nc.sync)
            nc.sync.dma_start(out=wo_sb, in_=woT[:, :, :])
            nc.gpsimd.dma_start(out=xvB[0], in_=xvT[:, 1, :, :])

            # later x loads stay on the gpsimd queue, gated behind the
            # critical phase so they don't steal HBM bandwidth from it.
            h1 = nc.gpsimd.dma_start(out=xvB[1], in_=xvT[:, 2, :, :])
            add_dep_helper(h1.ins, h_gate.ins, sync=True,
                           reason="late xv chunks wait for critical phase")

            def v_proj_tile(tt):
                ps = ps_mm.tile([P, GD], F32, tag="mm", name=f"psv{tt}")
                for kt in range(N_KT):
                    if tt < 4:
                        xs = xv0[:, kt, (tt % 4) * P:(tt % 4 + 1) * P]
                    else:
                        xs = xvB[tt // 4 - 1][:, kt, (tt % 4) * P:(tt % 4 + 1) * P]
                    nc.tensor.matmul(
                        ps,
                        lhsT=xs,
                        rhs=wv_t[kt],
                        start=(kt == 0),
                        stop=(kt == N_KT - 1),
                    )
                nc.vector.tensor_tensor(
                    out=vA[tt][:, :, VW - D_K:VW],
                    in0=ps.rearrange("p (h d) -> p h d", d=D_K),
                    in1=bv_sb.rearrange("p (h d) -> p h d", d=D_K),
                    op=ALU.add,
                )
                nc.vector.memset(vA[tt][:, :, 1:VW - D_K], 0.0)
                nc.vector.memset(vA[tt][:, :, 0:1], 1.0)

            def kq_part(w_t, b_sb, dsts, sc, nm, ncz, mt):
                xchs = xch_cache[(nm, ncz)]
                if nm == "q":
                    dsts[mt][ncz] = qT_pool.tile([P, NQ], BF16, tag="qT",
                                                 name=f"qT{mt}_{ncz}")
                ps = ps_mm.tile([P, NQ], F32, tag="mm", name=f"ps{nm}{ncz}{mt}")
                for kt in range(N_KT):
                    nc.tensor.matmul(
                        ps,
                        lhsT=w_t[kt][:, mt * P:(mt + 1) * P],
                        rhs=xchs[kt],
                        start=(kt == 0),
                        stop=(kt == N_KT - 1),
                    )
                nc.vector.tensor_scalar(
                    out=dsts[mt][ncz],
                    in0=ps,
                    scalar1=b_sb[:, mt:mt + 1],
                    scalar2=sc,
                    op0=ALU.add,
                    op1=ALU.mult,
                )

            # per-(mt, chunk) tiles so consumers unblock as soon as possible
            qTt = [[None for _ in range(N_QC)] for _ in range(N_MT)]
            kTt = [[const.tile([P, NQ], BF16, tag=f"kT{mt}_{ncz}", name=f"kT{mt}_{ncz}")
                    for ncz in range(N_QC)] for mt in range(N_MT)]
            # 128 cols per head: [ones, 63 zeros, 64 V-dims] so the AV output
            # puts the denominator at partition 0 and V rows at partition 64
            VW = 128
            vA = [const.tile([P, 2 * N_MT, VW], BF16, tag=f"v{tt}", name=f"v{tt}")
                  for tt in range(N_TT)]
            aoTq = [[const.tile([P, NQ], BF16, tag=f"ao{mt}_{qc}", name=f"ao{mt}_{qc}")
                     for qc in range(N_QC)] for mt in range(N_MT)]

            # ---- attention, software-pipelined: the AV pair for a key-block
            # is emitted two blocks after its scores/exp, so the PE (strict
            # FIFO queue) has scores work while ACT runs exp and never stalls
            # on a single exp's latency.
            pipe = {"q": []}

            def normalize(mt, qc, acc):
                for h2 in range(2):
                    rec1 = small_pool.tile([1, NQ], F32, tag="rec1",
                                           name=f"rec1{mt}_{qc}_{h2}")
                    nc.vector.reciprocal_approx_fast(
                        out=rec1, in_=acc[h2][0:1, :])
                    recb = small_pool.tile([1, NQ], BF16, tag="recb",
                                           name=f"recb{mt}_{qc}_{h2}")
                    nc.vector.tensor_copy(out=recb, in_=rec1)
                    # broadcast to partitions 64..127 via a K=1 outer product
                    # on the PE (partition-broadcast DMA is broken on HW, and
                    # a DRAM bounce costs ~3us of latency + 2 DMAs)
                    bc = ps_mm.tile([P, NQ], F32, tag="mm",
                                    name=f"bc{mt}_{qc}_{h2}")
                    nc.tensor.matmul(bc[VW - D_K:VW, :], lhsT=ones64[0:1, :],
                                     rhs=recb, start=True, stop=True)
                    nc.vector.tensor_tensor(
                        out=aoTq[mt][qc][h2 * D_K:(h2 + 1) * D_K, :],
                        in0=acc[h2][VW - D_K:VW, :],
                        in1=bc[VW - D_K:VW, :],
                        op=ALU.mult,
                    )

            def emit_av(st):
                mt, qc, kb, p3, av, acc = st
                nkb = 4 * qc + 4
                t = P * (kb - 4 * qc)
                grp_start = (kb % 8 == 0)
                grp_stop = (kb % 8 == 7) or (kb == nkb - 1)
                for h2 in range(2):
                    nc.tensor.matmul(
                        av[h2][:, max(t, 0):NQ],
                        lhsT=vA[kb][:, 2 * mt + h2, :],
                        rhs=p3[:, h2, max(t, 0):NQ],
                        start=grp_start,
                        stop=grp_stop,
                    )
                if grp_stop:  # evict group into SBUF accumulator
                    for h2 in range(2):
                        if kb < 8:
                            nc.vector.tensor_copy(out=acc[h2], in_=av[h2])
                        else:
                            nc.vector.tensor_tensor(
                                out=acc[h2], in0=acc[h2], in1=av[h2],
                                op=ALU.add,
                            )
                if kb == nkb - 1:
                    # pair complete: normalize here so the DVE queue sees the
                    # eviction before the ops that consume the accumulator.
                    normalize(mt, qc, acc)

            def attention_pair(mt, qc, interleave=None, warm=False):
                acc = [acc_pool.tile([VW, NQ], F32, tag="acc",
                                     name=f"acc{mt}_{qc}_{i}") for i in range(2)]
                av = [None, None]
                nkb = 4 * qc + 4
                for kb in range(nkb):
                    if kb % 8 == 0:
                        av = [ps_av.tile([VW, NQ], F32, tag="av",
                                         name=f"av{mt}_{qc}_{kb}_{i}")
                              for i in range(2)]
                    t = P * (kb - 4 * qc)  # <0 for full blocks
                    s_ps = ps_s.tile([P, 2 * NQ], F32, tag="s",
                                     name=f"s{mt}_{qc}_{kb}")
                    s3 = s_ps.rearrange("p (h n) -> p h n", n=NQ)
                    for h2 in range(2):
                        nc.tensor.matmul(
                            s3[:, h2, max(t, 0):NQ],
                            lhsT=kTt[mt][kb // 4][h2 * D_K:(h2 + 1) * D_K,
                                                 (kb % 4) * P:(kb % 4 + 1) * P],
                            rhs=qTt[mt][qc][h2 * D_K:(h2 + 1) * D_K,
                                            max(t, 0):NQ],
                            start=True,
                            stop=True,
                        )
                    pt = pt_pool.tile([P, 2 * NQ], BF16, tag="pt",
                                      name=f"pt{mt}_{qc}_{kb}")
                    p3 = pt.rearrange("p (h n) -> p h n", n=NQ)
                    if t <= 0:
                        nc.scalar.activation(out=pt, in_=s_ps, func=AF.Exp)
                    else:
                        nc.scalar.activation(out=p3[:, :, t:NQ],
                                             in_=s3[:, :, t:NQ], func=AF.Exp)
                    if t >= 0:  # diagonal sub-block: fused triangular mask
                        nc.vector.tensor_tensor(
                            out=p3[:, :, t:t + P],
                            in0=p3[:, :, t:t + P],
                            in1=mask_sb,
                            op=ALU.mult,
                        )
                    pipe["q"].append((mt, qc, kb, p3, av, acc))
                    if len(pipe["q"]) > 2:
                        emit_av(pipe["q"].pop(0))
                    if interleave is not None:
                        interleave(kb)

            def flush_av():
                while pipe["q"]:
                    emit_av(pipe["q"].pop(0))

            def o_proj_piece(qc, j, dc):
                lt = 4 * qc + j
                ps = ps_mm.tile([P, NQ], F32, tag="mm", name=f"po{lt}_{dc}")
                for kt in range(N_MT):
                    nc.tensor.matmul(
                        ps,
                        lhsT=aoTq[kt][qc][:, j * P:(j + 1) * P],
                        rhs=wo_sb[:, kt, dc * NQ:(dc + 1) * NQ],
                        start=(kt == 0),
                        stop=(kt == N_MT - 1),
                    )
                ot = osb_pool.tile([P, NQ], BF16, tag="ot", name=f"ot{lt}_{dc}")
                nc.vector.tensor_copy(out=ot, in_=ps)
                nc.sync.dma_start(
                    out=out[lt * P:(lt + 1) * P, dc * NQ:(dc + 1) * NQ],
                    in_=ot,
                )

            def park_piece(j, dc):
                # o_proj for the last wave, head-pairs 0..2 only; the kt=3
                # contribution joins after normalize(3,3) in the tail.
                pso = ps_mm.tile([P, NQ], F32, tag="mm", name=f"pop{j}_{dc}")
                for kt in range(3):
                    nc.tensor.matmul(
                        pso,
                        lhsT=aoTq[kt][3][:, j * P:(j + 1) * P],
                        rhs=wo_sb[:, kt, dc * NQ:(dc + 1) * NQ],
                        start=(kt == 0),
                        stop=(kt == 2),
                    )
                nc.vector.tensor_copy(out=park_sb[2 * j + dc], in_=pso)

            # ---- emission schedule.  Prologue: six chunk-0 chains run
            # kt-MAJOR (their matmuls interleaved slice-by-slice), borrowing
            # the idle ps_s/ps_av banks, so every arriving x-slice unblocks
            # six matmuls instead of one -- the strict-FIFO PE queue never
            # waits on one chain's slowest slice.  v_proj tiles (whose
            # wv/xv0 data lands last) interleave into pair (0,0).
            # k chains first (their data lands first), kt-major in the four
            # borrowed banks so every arriving half-tensor unblocks four
            # matmuls; then the v chains (next to arrive), then q kt-major.
            def four_kt_major(nm, pools):
                w_t = wk_t if nm == "k" else wq_t
                b_sb, dsts, sc = ((bk_sb, kTt, 1.0) if nm == "k"
                                  else (bq_sb, qTt, scale))
                pss = [pool.tile([P, NQ], F32, tag=tag, name=f"pc{nm}{mt}")
                       for mt, (pool, tag) in enumerate(pools)]
                for kt in range(N_KT):
                    for mt in range(N_MT):
                        nc.tensor.matmul(
                            pss[mt],
                            lhsT=w_t[kt][:, mt * P:(mt + 1) * P],
                            rhs=xch_cache[(nm, 0)][kt],
                            start=(kt == 0),
                            stop=(kt == N_KT - 1),
                        )
                for mt in range(N_MT):
                    if nm == "q":
                        dsts[mt][0] = qT_pool.tile([P, NQ], BF16, tag="qT",
                                                   name=f"qT{mt}_0")
                    nc.vector.tensor_scalar(
                        out=dsts[mt][0], in0=pss[mt],
                        scalar1=b_sb[:, mt:mt + 1], scalar2=sc,
                        op0=ALU.add, op1=ALU.mult,
                    )

            four_kt_major("k", [(ps_s, "s"), (ps_s, "s"),
                                (ps_av, "av"), (ps_av, "av")])
            for tt in range(4):
                v_proj_tile(tt)
            four_kt_major("q", [(ps_s, "s"), (ps_s, "s"),
                                (ps_av, "av"), (ps_av, "av")])

            def prologue_fill(kb):
                # chunk-1 projections: their data lands during pair (0,0)
                kq_part(wk_t, bk_sb, kTt, 1.0, "k", 1, kb)
                kq_part(wq_t, bq_sb, qTt, scale, "q", 1, kb)

            def wave3_fill(mt):
                # During the exp-bound last wave, interleave all deferred
                # o_proj work: pieces of chunk mt during pair (mt,3) for
                # mt<3, the kt0..2 partials of chunk 3 during pair (3,3).
                def fill(kb):
                    if kb % 2 == 1:
                        i = kb // 2
                        if mt < 3:
                            o_proj_piece(mt, i // 2, i % 2)
                        else:
                            park_piece(i // 2, i % 2)
                return fill

            for qc in range(N_QC):
                if 1 <= qc < 3:
                    kq_dmas("k", xkT, qc + 1, nc.gpsimd)
                    kq_dmas("q", xqT, qc + 1, nc.gpsimd)
                    if qc == 1:
                        nc.gpsimd.dma_start(out=xvB[2],
                                            in_=xvT[:, 3, :, :])
                for mt in range(N_MT):
                    if qc == 0 and mt == 0:
                        attention_pair(0, 0, interleave=prologue_fill,
                                       warm=True)
                    elif qc == 3:
                        attention_pair(mt, 3, interleave=wave3_fill(mt))
                    else:
                        attention_pair(mt, qc, warm=(qc == 0))
                    if qc == 0:
                        v_proj_tile(4 + mt)
                    elif qc < 3:
                        nz = qc + 1
                        kq_part(wk_t, bk_sb, kTt, 1.0, "k", nz, mt)
                        v_proj_tile(4 * nz + mt)
                        kq_part(wq_t, bq_sb, qTt, scale, "q", nz, mt)
                flush_av()
            # ---- tail: the kt=3 contribution of the last wave's o_proj
            # joins the parked kt0..2 partial via an identity-matmul inject
            # (PSUM accumulate), so the only post-PE work per piece is one
            # copy -- alternated DVE / ACT, both idle here.
            for j in range(4):
                for dc in range(2):
                    lt = 12 + j
                    pool = ps_av if (2 * j + dc) % 2 else ps_mm
                    ps2 = pool.tile([P, NQ], F32,
                                    tag="av" if (2 * j + dc) % 2 else "mm",
                                    name=f"pof{j}_{dc}")
                    nc.tensor.matmul(
                        ps2,
                        lhsT=aoTq[3][3][:, j * P:(j + 1) * P],
                        rhs=wo_sb[:, 3, dc * NQ:(dc + 1) * NQ],
                        start=True,
                        stop=False,
                    )
                    nc.tensor.matmul(
                        ps2,
                        lhsT=ident_sb,
                        rhs=park_sb[2 * j + dc],
                        start=False,
                        stop=True,
                    )
                    ot = osb_pool.tile([P, NQ], BF16, tag="ot",
                                       name=f"ot{lt}_{dc}")
                    if dc == 0:
                        nc.vector.tensor_copy(out=ot, in_=ps2)
                    else:
                        nc.scalar.copy(out=ot, in_=ps2)
                    nc.sync.dma_start(
                        out=out[lt * P:(lt + 1) * P, dc * NQ:(dc + 1) * NQ],
                        in_=ot,
                    )
    nc.finalize()
    return nc


def make_in_maps(Q, K, V, Wq, bq, Wk, bk, Wv, bv, Wo, bo, attn_mask=None):
    """Build the 8 per-core input maps from full (unsharded) inputs."""
    Q = np.asarray(Q, np.float32)
    K = np.asarray(K, np.float32)
    V = np.asarray(V, np.float32)
    Wq = np.asarray(Wq, np.float32)
    Wk = np.asarray(Wk, np.float32)
    Wv = np.asarray(Wv, np.float32)
    Wo = np.asarray(Wo, np.float32)
    bq = np.asarray(bq, np.float32)
    bk = np.asarray(bk, np.float32)
    bv = np.asarray(bv, np.float32)

    i_idx = np.arange(P)[:, None]
    j_idx = np.arange(P)[None, :]
    tri = (i_idx <= j_idx).astype(NPBF16)
    maskc = np.concatenate([tri, tri], axis=1)  # duplicated for the 2 heads
    ident = np.eye(P, dtype=NPBF16)

    def x4(A):
        # [D, L] -> [P, N_QC, N_KT, NQ], 8KB-contiguous per (p, chunk)
        return np.ascontiguousarray(
            A.reshape(N_KT, P, N_QC, NQ).transpose(1, 2, 0, 3)
        ).astype(NPBF16)

    def w3(W):
        # [D, GD] -> [P, N_KT, GD]
        return np.ascontiguousarray(
            W.reshape(N_KT, P, -1).transpose(1, 0, 2)
        ).astype(NPBF16)

    def wo3(W):
        # [GD, D] -> [P, N_MT, D]
        return np.ascontiguousarray(
            W.reshape(N_MT, P, -1).transpose(1, 0, 2)
        ).astype(NPBF16)

    xT = {}
    for b in range(B):
        xT[b] = tuple(x4(X[b].T) for X in (Q, K, V))
    grp = {}
    for g in range(TP):
        sl = slice(g * GD, (g + 1) * GD)
        grp[g] = dict(
            wqT=w3(Wq[sl, :].T),
            wkT=w3(Wk[sl, :].T),
            wvT=w3(Wv[sl, :].T),
            woT=wo3(Wo[:, sl].T),
            bq=np.ascontiguousarray(bq[sl].reshape(N_MT, P).T).astype(np.float32),
            bk=np.ascontiguousarray(bk[sl].reshape(N_MT, P).T).astype(np.float32),
            bv=np.ascontiguousarray(bv[sl].reshape(1, GD)).astype(np.float32),
        )
    in_maps = []
    for c in range(2 * B):
        b, g = c // 2, c % 2
        m = dict(grp[g])
        m["xqT"], m["xkT"], m["xvT"] = xT[b]
        m["maskc"] = maskc
        m["ident"] = ident
        in_maps.append(m)
    return in_maps


def assemble_output(results, bo):
    bo = np.asarray(bo, np.float32)
    out = np.empty((B, L, D_MODEL), np.float32)
    for b in range(B):
        out[b] = (results[2 * b]["out"].astype(np.float32)
                  + results[2 * b + 1]["out"].astype(np.float32) + bo)
    return out


_NC_CACHE = None


def kernel(**inputs) -> np.ndarray:
    global _NC_CACHE
    from concourse.bass_utils import run_bass_kernel_spmd

    if _NC_CACHE is None:
        _NC_CACHE = build_nc()
    in_maps = make_in_maps(**inputs)
    res = run_bass_kernel_spmd(_NC_CACHE, in_maps, core_ids=list(range(2 * B)))
    return assemble_output(res.results, inputs["bo"])
